# revision 1
# baseline (speedup 1.0000x reference)
"""Trainium2 Bass kernel for nn_C_Net_77807627534400 (sparse_attention).

Reference semantics: for each batch image and each class k in 1..11, the
per-class masked-normalized gray/rgb features form an [N,N] correlation,
softmax over the rgb-mask pixels, and a weighted mean of the rgb image is
written at the gray-mask pixels (if both masks have >= 2 pixels).

Because every pixel belongs to exactly one class, the 11 per-class [N,N]
matmuls fuse into ONE [N,N] matmul of per-class-normalized unit features.
Class matching is enforced by accumulating BIG * (rl^T @ gl) into the same
PSUM accumulation and using a constant exp bias of -(BIG+1): matching pairs
get exp(corr - 1) (corr in [-1,1] by Cauchy-Schwarz, so no overflow and no
row-max pass is needed); non-matching pairs get exp(corr - BIG - 1) == 0.

The matrix is computed transposed, Mt[j, i] (j = rgb pixel = partition), so
the softmax denominator and the [3,N] output are both plain PE matmuls over
j with no on-chip transpose of the attention matrix:
    O4[c,i] = sum_j img4[c,j] * exp(Mt[j,i] - BIG - 1),  img4 = [img; ones]
    out[i]  = rowvalid[i] ? O4[0:3,i] / max(O4[3,i], tiny) : -1

Sharding: 8 cores = 2 batches x 4 slices of 576 gray pixels. Each core
computes the full rgb-side normalization for its batch (redundant across 4
cores -- cheap) and its 576-column slice of the gray side.

Matmuls run as float32r (full PE rate). The BIR verifier requires fp32r
matmul operands to be *produced* as fp32r, so DMA-fed operand tensors are
declared float32r end-to-end (same bits as fp32) and compute-produced
operands (squares, units, exp, scaled labels, means) are written with
float32r output dtype. Small count/validity/broadcast matmuls that need
exact fp32 read the same tiles via bitcast.
"""

import numpy as np

import concourse.bass as bass
import concourse.tile as tile
from concourse import mybir
from concourse.bass_utils import run_bass_kernel_spmd
from concourse.vector_clock import ScopedClock

B, C, H, W, NCH = 2, 256, 48, 48, 12
N = H * W           # 2304
NK = NCH - 1        # classes 1..11
QS = 4              # gray-pixel slices per batch
NI = N // QS        # 576 rows per core
NCORES = B * QS     # 8
JC = N // 128       # 18 j-chunks
CC = C // 128       # 2 c-chunks
IW = 288            # i-chunk width (two per slice; >=256 keeps fp32r fast)
BIG = 128.0
F32 = mybir.dt.float32
F32R = mybir.dt.float32r
ALU = mybir.AluOpType
AF = mybir.ActivationFunctionType


class _TC(tile.TileContext):
    """Workaround: this walrus build rejects instructions carrying more than
    one sync-wait command. Split every multi-wait instruction into a chain of
    single-wait NOPs (same engine, program order preserved) followed by the
    original instruction holding the final wait."""

    def _add_instruction(self, inst):
        si = inst.sync_info
        if si is not None:
            waits = list(si.on_wait)
            if len(waits) > 1:
                nc = self.nc
                for w in waits[:-1]:
                    nop = mybir.InstNoOp(
                        name=nc.get_next_instruction_name(),
                        sync_info=mybir.SyncInfo(on_wait=[w], on_update=[]),
                        bass_nofuse=True,
                        engine=inst.engine,
                    )
                    super()._add_instruction(nop)
                si.on_wait = waits[-1:]
                inst.sync_info = si
        super()._add_instruction(inst)

    def _drain_and_barrier(self, tick_clock, wait_clock):
        nc = self.nc
        drain_inst = nc.sync.drain()
        wait_clock.add_sem_waits(
            drain_inst.ins, ScopedClock({None: tick_clock.global_clock})
        )
        si = drain_inst.ins.sync_info
        waits = list(si.on_wait) if si is not None else []
        if len(waits) > 1:
            si.on_wait = waits[:1]
            drain_inst.ins.sync_info = si
            for w in waits[1:]:
                extra = nc.sync.drain()
                extra.ins.sync_info = mybir.SyncInfo(on_wait=[w], on_update=[])

        nc.all_engine_barrier()
        assert self.sems is not None
        popped = nc._tile_sem_poison_stack.pop()
        assert popped is self._sem_poison
        nc.clear_and_free_semaphores(list(self.sems.allocated().values()))
        nc.all_engine_barrier()


def _f(ap):
    return ap.bitcast(F32)


def _build_nc():
    nc = bass.Bass(target_bir_lowering=False)

    d_rf = nc.dram_tensor("rf", [C, N], F32, kind="ExternalInput")
    d_rfT = nc.dram_tensor("rfT", [N, C + 2], F32R, kind="ExternalInput")
    d_gfT = nc.dram_tensor("gfT", [N, C + 2], F32R, kind="ExternalInput")
    d_gfs = nc.dram_tensor("gfs", [C, NI], F32, kind="ExternalInput")
    d_gls = nc.dram_tensor("gls", [NK, NI], F32R, kind="ExternalInput")
    d_rl = nc.dram_tensor("rl", [NK, N], F32R, kind="ExternalInput")
    d_glT = nc.dram_tensor("glT", [N, NK], F32R, kind="ExternalInput")
    d_rlT = nc.dram_tensor("rlT", [N, NK], F32R, kind="ExternalInput")
    d_imgT = nc.dram_tensor("imgT", [N, 4], F32R, kind="ExternalInput")
    d_ones = nc.dram_tensor("ones", [128, 128], F32R, kind="ExternalInput")
    d_out = nc.dram_tensor("out", [3, NI], F32, kind="ExternalOutput")

    with _TC(nc) as tc:
        with (
            tc.tile_pool(name="big", bufs=1) as big,
            tc.tile_pool(name="work", bufs=1) as work,
            tc.tile_pool(name="sq", bufs=1) as sqp,
            tc.tile_pool(name="expp", bufs=4) as expp,
            tc.tile_pool(name="small", bufs=1) as small,
            tc.tile_pool(name="psS", bufs=2, space="PSUM") as psS,
            tc.tile_pool(name="psM", bufs=2, space="PSUM") as psM,
            tc.tile_pool(name="psO", bufs=1, space="PSUM") as psO,
        ):
            # ---- loads ----
            # ordered by first consumer; the big transposed-feature loads are
            # split so the class-means matmuls start behind the first piece
            s_ones = big.tile([128, 128], F32R)
            nc.sync.dma_start(s_ones[:], d_ones[:])
            s_glT = big.tile([128, JC, NK], F32R)
            nc.sync.dma_start(s_glT[:], d_glT.rearrange("(a p) k -> p a k", p=128))
            s_gfT = big.tile([128, JC, C + 2], F32R)
            gfT_r = d_gfT.rearrange("(a p) c -> p a c", p=128)
            for piece in range(0, JC, 3):
                nc.sync.dma_start(s_gfT[:, piece:piece + 3, :],
                                  gfT_r[:, piece:piece + 3, :])
            s_gls = big.tile([NK, NI], F32R)
            nc.sync.dma_start(s_gls[:], d_gls[:])
            s_rlT = big.tile([128, JC, NK], F32R)
            nc.sync.dma_start(s_rlT[:], d_rlT.rearrange("(a p) k -> p a k", p=128))
            s_rfT = big.tile([128, JC, C + 2], F32R)
            rfT_r = d_rfT.rearrange("(a p) c -> p a c", p=128)
            for piece in range(0, JC, 3):
                nc.sync.dma_start(s_rfT[:, piece:piece + 3, :],
                                  rfT_r[:, piece:piece + 3, :])
            s_gfs = []
            for cc in range(CC):
                t = big.tile([128, NI], F32, tag=f"gfs{cc}", name=f"gfs{cc}")
                nc.sync.dma_start(t[:], d_gfs[cc * 128:(cc + 1) * 128, :])
                s_gfs.append(t)
            s_rl = big.tile([NK, N], F32R)
            nc.sync.dma_start(s_rl[:], d_rl[:])
            s_rf = []
            for cc in range(CC):
                t = big.tile([128, N], F32, tag=f"rf{cc}", name=f"rf{cc}")
                nc.sync.dma_start(t[:], d_rf[cc * 128:(cc + 1) * 128, :])
                s_rf.append(t)
            s_imgT = big.tile([128, JC, 4], F32R)
            nc.sync.dma_start(s_imgT[:], d_imgT.rearrange("(a p) k -> p a k", p=128))

            # bias constants for non-Copy activations (const-AP pool is
            # not populated in this flow, so pass explicit per-partition APs)
            b_zero = big.tile([128, 1], F32)
            nc.vector.memset(b_zero[:], 0.0)
            b_eps = big.tile([128, 1], F32)
            nc.vector.memset(b_eps[:], 1e-12)
            b_exp = big.tile([128, 1], F32)
            nc.vector.memset(b_exp[:], -(BIG + 1.0))
            b_neg1 = big.tile([128, 1], F32)
            nc.vector.memset(b_neg1[:], -1.0)

            # ---- per-class sums + counts in one accumulation:
            # rhs carries [features | ones], so column C of the sums is cnt
            def class_means(s_lT, s_fT, nmtag):
                ps = psS.tile([NK, C + 2], F32, tag="t", name=f"ps_mean{nmtag}")
                for jc in range(JC):
                    nc.tensor.matmul(ps[:], s_lT[:, jc, :], s_fT[:, jc, :],
                                     start=(jc == 0), stop=(jc == JC - 1))
                cnt = small.tile([NK, 1], F32, name=f"cnt{nmtag}")
                nc.scalar.copy(cnt[:], ps[:, C:C + 1])
                rc = small.tile([NK, 1], F32, name=f"rc{nmtag}")
                nc.vector.tensor_scalar(rc[:], cnt[:], 1.0, None, ALU.max)
                nc.vector.reciprocal(rc[:], rc[:])
                meanT = work.tile([NK, C], F32R, name=f"mean{nmtag}")
                nc.scalar.activation(meanT[:], ps[:, 0:C], AF.Copy,
                                     bias=0.0, scale=rc[:])
                return meanT, cnt

            meanT_g, cnt_g = class_means(s_glT, s_gfT, "g")
            meanT_r, cnt_r = class_means(s_rlT, s_rfT, "r")
            vg = small.tile([NK, 1], F32)
            nc.vector.tensor_scalar(vg[:], cnt_g[:], 1.5, None, ALU.is_gt)
            valid = small.tile([NK, 1], F32)
            nc.vector.tensor_scalar(valid[:], cnt_r[:], 1.5, None, ALU.is_gt)
            nc.vector.tensor_mul(valid[:], valid[:], vg[:])
            valid3 = small.tile([NK, 3], F32)
            for i in range(3):
                nc.vector.tensor_copy(valid3[:, i:i + 1], valid[:])

            # mask weights: BIG * rl (early: only needs the rl load)
            s_rlB = big.tile([NK, N], F32R)
            nc.scalar.mul(s_rlB[:], _f(s_rl[:]), BIG)

            # ---- gray-side normalize: unit_g = (gf - mu) / ||gf - mu|| ----
            # (emitted first; the whole main loop needs unit_g)
            unit_g = [work.tile([128, NI], F32R, tag=f"unitg{cc}",
                                name=f"unitg{cc}")
                      for cc in range(CC)]
            for ib in range(2):
                j0 = ib * IW
                sl = slice(j0, j0 + IW)
                barg = [sqp.tile([128, IW], F32, tag=f"barg{cc}", bufs=2,
                                 name=f"barg{cc}")
                        for cc in range(CC)]
                sqg = [sqp.tile([128, IW], F32R, tag=f"sqg{cc}", bufs=2,
                                name=f"sqg{cc}")
                       for cc in range(CC)]
                for cc in range(CC):
                    ps = psS.tile([128, 512], F32, tag="t", name="ps_mug")
                    nc.tensor.matmul(ps[:, 0:IW],
                                     meanT_g[:, cc * 128:(cc + 1) * 128],
                                     s_gls[:, sl], start=True, stop=True)
                    nc.vector.tensor_sub(barg[cc][:], s_gfs[cc][:, sl],
                                         ps[:, 0:IW])
                    if cc == 0:
                        nc.scalar.activation(sqg[cc][:], barg[cc][:],
                                             AF.Square, bias=b_zero[:])
                    else:
                        nc.vector.tensor_mul(sqg[cc][:], barg[cc][:],
                                             barg[cc][:])
                ps = psS.tile([128, 512], F32, tag="t", name="ps_ssqg")
                for cc in range(CC):
                    nc.tensor.matmul(ps[:, 0:IW], s_ones[:], sqg[cc][:],
                                     start=(cc == 0), stop=(cc == CC - 1))
                nc.scalar.activation(ps[:, 0:IW], ps[:, 0:IW],
                                     AF.Sqrt, bias=b_eps[:])
                rbg = sqp.tile([128, IW], F32, tag="rbg", bufs=2, name="rbg")
                nc.vector.reciprocal(rbg[:], ps[:, 0:IW])
                for cc in range(CC):
                    nc.vector.tensor_mul(unit_g[cc][:, sl], barg[cc][:],
                                         rbg[:])

            # ---- rgb-side normalize in 256-wide chunks, interleaved with
            # the main attention loop: chunk ib yields unit_r columns for
            # exactly j-chunks 2*ib and 2*ib+1, so PE starts attention
            # matmuls while later chunks are still normalizing. ----
            RW = 256
            NRC = N // RW          # 9 chunks
            ps_O4 = psO.tile([4, 2, 512], F32)

            def attention_jc(jc, ur):
                # ur: this chunk's unit_r tiles [128, RW]; jc covers
                # columns [jc*128, jc*128+128) => local offset (jc%2)*128
                lo = (jc % 2) * 128
                j0 = jc * 128
                ps_mt = psM.tile([128, 2, 512], F32, tag="mt", name="ps_mt")
                for ic in range(2):
                    i0 = ic * IW
                    nc.tensor.matmul(ps_mt[:, ic, 0:IW],
                                     ur[0][:, lo:lo + 128],
                                     unit_g[0][:, i0:i0 + IW],
                                     start=True, stop=False)
                    nc.tensor.matmul(ps_mt[:, ic, 0:IW],
                                     ur[1][:, lo:lo + 128],
                                     unit_g[1][:, i0:i0 + IW],
                                     start=False, stop=False)
                    nc.tensor.matmul(ps_mt[:, ic, 0:IW],
                                     s_rlB[:, j0:j0 + 128],
                                     s_gls[:, i0:i0 + IW],
                                     start=False, stop=True)
                s_exp = expp.tile([128, NI], F32R, tag="exp", name="s_exp")
                nc.scalar.activation(
                    s_exp[:].rearrange("p (a b) -> p a b", a=2),
                    ps_mt[:, :, 0:IW], AF.Exp, bias=b_exp[:])
                for ic in range(2):
                    i0 = ic * IW
                    nc.tensor.matmul(ps_O4[:, ic, 0:IW],
                                     s_imgT[:, jc, :],
                                     s_exp[:, i0:i0 + IW],
                                     start=(jc == 0), stop=(jc == JC - 1))

            rc_tiles = {}

            def r_chunk(ib):
                j0 = ib * RW
                sl = slice(j0, j0 + RW)
                barr = [sqp.tile([128, RW], F32, tag=f"barr{cc}", bufs=4,
                                 name=f"barr{cc}")
                        for cc in range(CC)]
                sqr = [sqp.tile([128, RW], F32R, tag=f"sqr{cc}", bufs=4,
                                name=f"sqr{cc}")
                       for cc in range(CC)]
                ur = [sqp.tile([128, RW], F32R, tag=f"ur{cc}", bufs=4,
                               name=f"ur{cc}")
                      for cc in range(CC)]
                for cc in range(CC):
                    ps = psS.tile([128, 512], F32, tag="t", name="ps_mur")
                    nc.tensor.matmul(ps[:, 0:RW],
                                     meanT_r[:, cc * 128:(cc + 1) * 128],
                                     s_rl[:, sl], start=True, stop=True)
                    nc.vector.tensor_sub(barr[cc][:], s_rf[cc][:, sl],
                                         ps[:, 0:RW])
                    if cc == 0:
                        nc.scalar.activation(sqr[cc][:], barr[cc][:],
                                             AF.Square, bias=b_zero[:])
                    else:
                        nc.vector.tensor_mul(sqr[cc][:], barr[cc][:],
                                             barr[cc][:])
                ps = psS.tile([128, 512], F32, tag="t", name="ps_ssqr")
                for cc in range(CC):
                    nc.tensor.matmul(ps[:, 0:RW], s_ones[:], sqr[cc][:],
                                     start=(cc == 0), stop=(cc == CC - 1))
                nc.scalar.activation(ps[:, 0:RW], ps[:, 0:RW],
                                     AF.Sqrt, bias=b_eps[:])
                rbr = sqp.tile([128, RW], F32, tag="rbr", bufs=4, name="rbr")
                nc.vector.reciprocal(rbr[:], ps[:, 0:RW])
                for cc in range(CC):
                    nc.vector.tensor_mul(ur[cc][:], barr[cc][:], rbr[:])
                rc_tiles[ib] = ur

            r_chunk(0)
            r_chunk(1)

            for ib in range(NRC):
                ur = rc_tiles.pop(ib)
                attention_jc(2 * ib, ur)
                attention_jc(2 * ib + 1, ur)
                if ib + 2 < NRC:
                    r_chunk(ib + 2)

            # ---- finalize: divide by row-sum, apply validity, write out ----
            s_O4 = small.tile([4, NI], F32)
            nc.scalar.copy(s_O4[:].rearrange("p (a b) -> p a b", a=2),
                           ps_O4[:, :, 0:IW])
            # compute engines need partition starts in {0,32,64,96}; move the
            # rowsum row to partition 0 with a tiny SBUF->SBUF DMA first
            s_rs = small.tile([1, NI], F32)
            nc.sync.dma_start(s_rs[:], s_O4[3:4, :])
            s_rcp = small.tile([1, NI], F32)
            nc.vector.tensor_scalar(s_rcp[:], s_rs[:], 1e-30, None, ALU.max)
            nc.vector.reciprocal(s_rcp[:], s_rcp[:])
            s_res = small.tile([3, NI], F32)
            for ic in range(2):
                i0 = ic * IW
                ps_r3 = psS.tile([3, 512], F32, tag="t", name="ps_r3")
                nc.tensor.matmul(ps_r3[:, 0:IW], _f(s_ones[0:1, 0:3]),
                                 s_rcp[:, i0:i0 + IW], start=True, stop=True)
                # Od = O4 * recip(rowsum)
                nc.vector.tensor_mul(s_res[:, i0:i0 + IW],
                                     s_O4[0:3, i0:i0 + IW], ps_r3[:, 0:IW])
            for ic in range(2):
                i0 = ic * IW
                ps_rv = psS.tile([3, 512], F32, tag="t", name="ps_rv")
                nc.tensor.matmul(ps_rv[:, 0:IW], valid3[:],
                                 _f(s_gls[:, i0:i0 + IW]),
                                 start=True, stop=True)
                # out = (Od + 1) * rowvalid - 1  (exact select for rv in {0,1})
                nc.vector.scalar_tensor_tensor(
                    s_res[:, i0:i0 + IW], s_res[:, i0:i0 + IW], 1.0,
                    ps_rv[:, 0:IW], ALU.add, ALU.mult)
            nc.scalar.add(s_res[:], s_res[:], b_neg1[0:3, :])
            nc.sync.dma_start(d_out[:], s_res[:])

    return nc


_NC_CACHE = None


def _get_nc():
    global _NC_CACHE
    if _NC_CACHE is None:
        _NC_CACHE = _build_nc()
    return _NC_CACHE


def build_in_maps(gray_feature, rgb_feature, rgb_image, gray_label, rgb_label):
    gf_all = np.ascontiguousarray(gray_feature, dtype=np.float32).reshape(B, C, N)
    rf_all = np.ascontiguousarray(rgb_feature, dtype=np.float32).reshape(B, C, N)
    img_all = np.ascontiguousarray(rgb_image, dtype=np.float32).reshape(B, 3, N)
    gl_all = np.ascontiguousarray(gray_label, dtype=np.float32).reshape(B, NCH, N)
    rl_all = np.ascontiguousarray(rgb_label, dtype=np.float32).reshape(B, NCH, N)

    ones = np.ones((128, 128), np.float32)
    in_maps = []
    for core in range(NCORES):
        b, q = divmod(core, QS)
        sl = slice(q * NI, (q + 1) * NI)
        gf = gf_all[b]
        rf = rf_all[b]
        gl = gl_all[b][1:]
        rl = rl_all[b][1:]
        img4 = np.concatenate([img_all[b], np.ones((1, N), np.float32)], 0)
        in_maps.append({
            "rf": rf,
            "rfT": np.ascontiguousarray(
                np.concatenate([rf, np.ones((1, N), np.float32),
                                np.zeros((1, N), np.float32)], 0).T),
            "gfT": np.ascontiguousarray(
                np.concatenate([gf, np.ones((1, N), np.float32),
                                np.zeros((1, N), np.float32)], 0).T),
            "gfs": np.ascontiguousarray(gf[:, sl]),
            "gls": np.ascontiguousarray(gl[:, sl]),
            "rl": rl,
            "glT": np.ascontiguousarray(gl.T),
            "rlT": np.ascontiguousarray(rl.T),
            "imgT": np.ascontiguousarray(img4.T),
            "ones": ones,
        })
    return in_maps


def kernel(gray_feature, rgb_feature, rgb_image, gray_label, rgb_label):
    in_maps = build_in_maps(gray_feature, rgb_feature, rgb_image,
                            gray_label, rgb_label)
    res = run_bass_kernel_spmd(_get_nc(), in_maps, list(range(NCORES)))

    canvas = np.empty((B, 3, N), np.float32)
    for core in range(NCORES):
        b, q = divmod(core, QS)
        canvas[b, :, q * NI:(q + 1) * NI] = res.results[core]["out"]
    return canvas.reshape(B, 3, H, W)



# revision 10
# speedup vs baseline: 1.8019x; 1.8019x over previous
"""Trainium2 Bass kernel for nn_C_Net_77807627534400 (sparse_attention).

Reference semantics: for each batch image and each class k in 1..11, the
per-class masked-normalized gray/rgb features form an [N,N] correlation,
softmax over the rgb-mask pixels, and a weighted mean of the rgb image is
written at the gray-mask pixels (if both masks have >= 2 pixels).

Because every pixel belongs to exactly one class, the 11 per-class [N,N]
matmuls fuse into ONE [N,N] matmul of per-class-centered features. The
class-match mask is enforced EXACTLY in the output matmul: expand img4
(rgb + ones row) to 44 rows IMG4R[(c,k), j] = img4[c,j] * rl[k,j], so

    O4K[(c,k), i] = sum_j img4[c,j] rl[k,j] e[j,i]
    O4[c, i]      = sum_k gl[k,i] O4K[(c,k), i]     (per-i class select)

with e[j,i] = exp(corr[j,i] - 1) computed WITHOUT any masking bias. The
collapse is one elementwise multiply by gl44 (gl broadcast to 44 rows via a
tiny matmul) plus a [44 -> 4] summing matmul. Numerator rows c=0..2 and the
softmax denominator (c=3, the ones row) come out of the same accumulation.

Normalization: gray side is explicitly normalized (unit_g); the rgb side is
NOT -- the raw centered bar_r is the matmul operand and 1/||bar_r_j|| is
applied as the per-partition *scale* of the Exp activation. Per-j sumsq is
computed directly in j-partition layout with tiny N=1 matmuls
(sum_c bar^2 = sq_chunk^T @ ones_col). rsqrt is exp(-0.5*ln(x)): ScalarE
uses ONLY the natural_log_exp_and_others table set -- no table thrash.

Dtypes: fp16 operands for all large matmuls (full PE rate, FWL weight
loads), fp8e4 for the transposed features/labels that feed the class-means
accumulation (precision is ample: the means average ~450 pixels). PSUM is
fp32 throughout. All DRAM tensors are host-side laid out to exactly match
their SBUF tiles so every DMA is contiguous.

Sharding: 8 cores = 2 batches x 4 slices of 576 gray pixels. Each core
computes the full rgb side for its batch (redundant across 4 cores) and its
576-column slice of the gray side.
"""

import numpy as np

import concourse.bass as bass
import concourse.tile as tile
from concourse import mybir
from concourse.bass_utils import run_bass_kernel_spmd
from concourse.vector_clock import ScopedClock

B, C, H, W, NCH = 2, 256, 48, 48, 12
N = H * W           # 2304
NK = NCH - 1        # classes 1..11
QS = 4              # gray-pixel slices per batch
NI = N // QS        # 576 rows per core
NCORES = B * QS     # 8
JC = N // 128       # 18 j-chunks
CC = C // 128       # 2 c-chunks
IW = 288            # i-chunk width (two per slice)
RW = 256            # rgb normalize chunk width
NRC = N // RW       # 9 rgb chunks
M4 = 4 * NK         # 44 expanded img rows
F32 = mybir.dt.float32
F32R = mybir.dt.float32r
F16 = mybir.dt.float16
F8 = mybir.dt.float8e4
ALU = mybir.AluOpType
AF = mybir.ActivationFunctionType


class _TC(tile.TileContext):
    """Workaround: this walrus build rejects instructions carrying more than
    one sync-wait command. Split every multi-wait instruction into a chain of
    single-wait NOPs (same engine, program order preserved) followed by the
    original instruction holding the final wait."""

    def _add_instruction(self, inst):
        si = inst.sync_info
        if si is not None:
            waits = list(si.on_wait)
            if len(waits) > 1:
                nc = self.nc
                for w in waits[:-1]:
                    nop = mybir.InstNoOp(
                        name=nc.get_next_instruction_name(),
                        sync_info=mybir.SyncInfo(on_wait=[w], on_update=[]),
                        bass_nofuse=True,
                        engine=inst.engine,
                    )
                    super()._add_instruction(nop)
                si.on_wait = waits[-1:]
                inst.sync_info = si
        super()._add_instruction(inst)

    def _drain_and_barrier(self, tick_clock, wait_clock):
        nc = self.nc
        drain_inst = nc.sync.drain()
        wait_clock.add_sem_waits(
            drain_inst.ins, ScopedClock({None: tick_clock.global_clock})
        )
        si = drain_inst.ins.sync_info
        waits = list(si.on_wait) if si is not None else []
        if len(waits) > 1:
            si.on_wait = waits[:1]
            drain_inst.ins.sync_info = si
            for w in waits[1:]:
                extra = nc.sync.drain()
                extra.ins.sync_info = mybir.SyncInfo(on_wait=[w], on_update=[])

        nc.all_engine_barrier()
        assert self.sems is not None
        popped = nc._tile_sem_poison_stack.pop()
        assert popped is self._sem_poison
        nc.clear_and_free_semaphores(list(self.sems.allocated().values()))
        nc.all_engine_barrier()


def _build_nc():
    nc = bass.Bass(target_bir_lowering=False)

    d_glT = nc.dram_tensor("glT8", [128, JC, NK], F8, kind="ExternalInput")
    d_gfT = nc.dram_tensor("gfT8", [128, JC, 258], F8, kind="ExternalInput")
    d_rlT = nc.dram_tensor("rlT8", [128, JC, NK], F8, kind="ExternalInput")
    d_rfT = nc.dram_tensor("rfT8", [128, JC, 258], F8, kind="ExternalInput")
    d_gls = nc.dram_tensor("gls", [NK, NI], F16, kind="ExternalInput")
    d_gfs = nc.dram_tensor("gfs", [128, CC, NI], F16, kind="ExternalInput")
    d_rl = nc.dram_tensor("rl", [NK, N], F16, kind="ExternalInput")
    d_rf = nc.dram_tensor("rf", [128, CC, N], F16, kind="ExternalInput")
    d_i4r = nc.dram_tensor("i4r", [128, JC, M4], F16, kind="ExternalInput")
    d_kc = nc.dram_tensor("kc", [NK, M4], F16, kind="ExternalInput")
    d_cs = nc.dram_tensor("cs", [M4, 4], F16, kind="ExternalInput")
    d_out = nc.dram_tensor("out", [3, NI], F32, kind="ExternalOutput")

    with _TC(nc) as tc:
        with (
            tc.tile_pool(name="big", bufs=1) as big,
            tc.tile_pool(name="work", bufs=1) as work,
            tc.tile_pool(name="chk", bufs=6) as chk,
            tc.tile_pool(name="expp", bufs=4) as expp,
            tc.tile_pool(name="small", bufs=1) as small,
            tc.tile_pool(name="psS", bufs=2, space="PSUM") as psS,
            tc.tile_pool(name="psM", bufs=2, space="PSUM") as psM,
            tc.tile_pool(name="psO", bufs=1, space="PSUM") as psO,
        ):
            # ---- loads, in consumption order ----
            s_glT = big.tile([128, JC, NK], F8)
            nc.sync.dma_start(s_glT[:], d_glT[:])
            s_gfT = big.tile([128, JC, 258], F8)
            for p in range(0, JC, 6):
                nc.sync.dma_start(s_gfT[:, p:p + 6, :], d_gfT[:, p:p + 6, :])
            s_rlT = big.tile([128, JC, NK], F8)
            nc.sync.dma_start(s_rlT[:], d_rlT[:])
            s_rfT = big.tile([128, JC, 258], F8)
            for p in range(0, JC, 6):
                nc.sync.dma_start(s_rfT[:, p:p + 6, :], d_rfT[:, p:p + 6, :])
            s_gls = big.tile([NK, NI], F16)
            nc.sync.dma_start(s_gls[:], d_gls[:])
            s_gfs = big.tile([128, CC, NI], F16)
            nc.sync.dma_start(s_gfs[:], d_gfs[:])
            s_rl = big.tile([NK, N], F16)
            nc.sync.dma_start(s_rl[:], d_rl[:])
            s_rf = big.tile([128, CC, N], F16)
            for p in range(0, N, 576):
                nc.sync.dma_start(s_rf[:, :, p:p + 576], d_rf[:, :, p:p + 576])
            s_i4r = big.tile([128, JC, M4], F16)
            for p in range(0, JC, 9):
                nc.sync.dma_start(s_i4r[:, p:p + 9, :], d_i4r[:, p:p + 9, :])
            s_kc = big.tile([NK, M4], F16)
            nc.sync.dma_start(s_kc[:], d_kc[:])
            s_cs = big.tile([M4, 4], F16)
            nc.sync.dma_start(s_cs[:], d_cs[:])

            # on-chip constants
            s_ones = big.tile([128, 128], F16)
            nc.vector.memset(s_ones[:], 1.0)
            b_zero = big.tile([128, 1], F32)
            nc.vector.memset(b_zero[:], 0.0)
            b_eps = big.tile([128, 1], F32)
            nc.vector.memset(b_eps[:], 1e-12)
            b_neg1 = big.tile([128, 1], F32)
            nc.vector.memset(b_neg1[:], -1.0)

            # ---- per-class sums + counts (col 256 is the ones column) ----
            def class_means(s_lT, s_fT, nmtag):
                ps = psS.tile([NK, 512], F32, tag="t", name=f"ps_mean{nmtag}")
                for jc in range(JC):
                    nc.tensor.matmul(ps[:, 0:258], s_lT[:, jc, :],
                                     s_fT[:, jc, :],
                                     start=(jc == 0), stop=(jc == JC - 1))
                cnt = small.tile([NK, 1], F32, name=f"cnt{nmtag}")
                nc.vector.tensor_copy(cnt[:], ps[:, 256:257])
                rc = small.tile([NK, 1], F32, name=f"rc{nmtag}")
                nc.vector.tensor_scalar(rc[:], cnt[:], 1.0, None, ALU.max)
                nc.vector.reciprocal(rc[:], rc[:])
                meanT = work.tile([NK, C], F16, name=f"mean{nmtag}")
                nc.scalar.activation(meanT[:], ps[:, 0:C], AF.Copy,
                                     bias=0.0, scale=rc[:])
                return meanT, cnt

            meanT_g, cnt_g = class_means(s_glT, s_gfT, "g")
            meanT_r, cnt_r = class_means(s_rlT, s_rfT, "r")
            vg = small.tile([NK, 1], F32)
            nc.vector.tensor_scalar(vg[:], cnt_g[:], 1.5, None, ALU.is_gt)
            valid = small.tile([NK, 1], F32)
            nc.vector.tensor_scalar(valid[:], cnt_r[:], 1.5, None, ALU.is_gt)
            nc.vector.tensor_mul(valid[:], valid[:], vg[:])
            valid3 = small.tile([NK, 3], F16)
            for i in range(3):
                nc.vector.tensor_copy(valid3[:, i:i + 1], valid[:])

            # gl44[(c,k), i] = gl[k, i] for the final class-collapse
            s_gl44 = small.tile([M4, NI], F16)
            for h in range(2):
                sl = slice(h * IW, (h + 1) * IW)
                ps = psS.tile([M4, 512], F32, tag="t", name="ps_gl44")
                nc.tensor.matmul(ps[:, 0:IW], s_kc[:], s_gls[:, sl],
                                 start=True, stop=True)
                nc.scalar.activation(s_gl44[:, sl], ps[:, 0:IW], AF.Copy,
                                     bias=0.0)

            # ---- gray side: unit_g = (gf - mu) / ||gf - mu|| ----
            unit_g = work.tile([128, CC, NI], F16, name="unitg")
            for ib in range(2):
                sl = slice(ib * IW, (ib + 1) * IW)
                barg = [chk.tile([128, IW], F16, tag=f"barg{cc}", bufs=2,
                                 name=f"barg{cc}") for cc in range(CC)]
                sqg = [chk.tile([128, IW], F16, tag=f"sqg{cc}", bufs=2,
                                name=f"sqg{cc}") for cc in range(CC)]
                for cc in range(CC):
                    ps = psS.tile([128, 512], F32, tag="t", name="ps_mug")
                    nc.tensor.matmul(ps[:, 0:IW],
                                     meanT_g[:, cc * 128:(cc + 1) * 128],
                                     s_gls[:, sl], start=True, stop=True)
                    nc.vector.tensor_sub(barg[cc][:], s_gfs[:, cc, sl],
                                         ps[:, 0:IW])
                    nc.vector.tensor_mul(sqg[cc][:], barg[cc][:], barg[cc][:])
                ps2 = psS.tile([128, 512], F32, tag="t", name="ps_ssqg")
                for cc in range(CC):
                    nc.tensor.matmul(ps2[:, 0:IW], s_ones[:], sqg[cc][:],
                                     start=(cc == 0), stop=(cc == CC - 1))
                lng = chk.tile([128, IW], F32, tag="lng", bufs=2, name="lng")
                nc.scalar.activation(lng[:], ps2[:, 0:IW], AF.Ln,
                                     bias=b_eps[:])
                rbg = chk.tile([128, IW], F32, tag="rbg", bufs=2, name="rbg")
                nc.scalar.activation(rbg[:], lng[:], AF.Exp,
                                     bias=b_zero[:], scale=-0.5)
                for cc in range(CC):
                    nc.vector.tensor_mul(unit_g[:, cc, sl], barg[cc][:],
                                         rbg[:])

            # ---- rgb side: bar_r chunks + per-j sumsq (j-partition layout).
            # 1/||bar_r_j|| is applied later as the Exp per-partition scale.
            bar_r = {}

            def r_chunk(ib):
                sl = slice(ib * RW, (ib + 1) * RW)
                barr = [chk.tile([128, RW], F16, tag=f"barr{cc}", bufs=8,
                                 name=f"barr{cc}") for cc in range(CC)]
                sqr = [chk.tile([128, RW], F16, tag=f"sqr{cc}", bufs=4,
                                name=f"sqr{cc}") for cc in range(CC)]
                for cc in range(CC):
                    ps = psS.tile([128, 512], F32, tag="t", name="ps_mur")
                    nc.tensor.matmul(ps[:, 0:RW],
                                     meanT_r[:, cc * 128:(cc + 1) * 128],
                                     s_rl[:, sl], start=True, stop=True)
                    nc.vector.tensor_sub(barr[cc][:], s_rf[:, cc, sl],
                                         ps[:, 0:RW])
                    nc.vector.tensor_mul(sqr[cc][:], barr[cc][:], barr[cc][:])
                for h in range(2):
                    jc = 2 * ib + h
                    ps = psS.tile([128, 512], F32, tag="t", name="ps_ssqr")
                    for cc in range(CC):
                        nc.tensor.matmul(ps[:, 0:1],
                                         sqr[cc][:, h * 128:(h + 1) * 128],
                                         s_ones[:, 0:1],
                                         start=(cc == 0), stop=(cc == CC - 1))
                    bi, col = (0, jc) if jc < 8 else (1, jc - 8)
                    nc.vector.tensor_copy(ssq[bi][:, col:col + 1], ps[:, 0:1])
                bar_r[ib] = barr

            # per-j sumsq staging + rsqrt batches: A covers jc 0..7
            # (chunks 0..3), B covers jc 8..17 (chunks 4..8)
            ssq = [small.tile([128, 8], F32, name="ssqA"),
                   small.tile([128, 10], F32, name="ssqB")]
            rsq = [small.tile([128, 8], F32, name="rsqA"),
                   small.tile([128, 10], F32, name="rsqB")]

            def rsqrt_batch(bi):
                t = small.tile([128, 10], F32, name=f"lnr{bi}")
                w = ssq[bi].shape[1]
                nc.scalar.activation(t[:, 0:w], ssq[bi][:], AF.Ln,
                                     bias=b_eps[:])
                nc.scalar.activation(rsq[bi][:], t[:, 0:w], AF.Exp,
                                     bias=b_zero[:], scale=-0.5)

            for ib in range(4):
                r_chunk(ib)
            rsqrt_batch(0)

            # ---- attention + masked-output accumulation ----
            ps_O4K = psO.tile([M4, 2, 512], F32)

            def attention_jc(jc):
                ib, h = jc // 2, jc % 2
                lo = h * 128
                barr = bar_r[ib]
                ps_mt = psM.tile([128, 2, 512], F32, tag="mt", name="ps_mt")
                for ic in range(2):
                    i0 = ic * IW
                    nc.tensor.matmul(ps_mt[:, ic, 0:IW],
                                     barr[0][:, lo:lo + 128],
                                     unit_g[:, 0, i0:i0 + IW],
                                     start=True, stop=False)
                    nc.tensor.matmul(ps_mt[:, ic, 0:IW],
                                     barr[1][:, lo:lo + 128],
                                     unit_g[:, 1, i0:i0 + IW],
                                     start=False, stop=True)
                bi, col = (0, jc) if jc < 8 else (1, jc - 8)
                s_exp = expp.tile([128, NI], F16, tag="exp", name="s_exp")
                nc.scalar.activation(
                    s_exp[:].rearrange("p (a b) -> p a b", a=2),
                    ps_mt[:, :, 0:IW], AF.Exp, bias=b_neg1[:],
                    scale=rsq[bi][:, col:col + 1])
                for ic in range(2):
                    i0 = ic * IW
                    nc.tensor.matmul(ps_O4K[:, ic, 0:IW], s_i4r[:, jc, :],
                                     s_exp[:, i0:i0 + IW],
                                     start=(jc == 0), stop=(jc == JC - 1))

            # chunks 4..8 are emitted during attention jc 0..4 so that the
            # second rsqrt batch (consumed from jc=8) is emitted by jc=4
            for jc in range(JC):
                attention_jc(jc)
                if jc < 5:
                    r_chunk(jc + 4)
                    if jc == 4:
                        rsqrt_batch(1)

            # ---- finalize: class-collapse, divide by row-sum, validity ----
            prod = small.tile([M4, NI], F16)
            nc.vector.tensor_mul(prod[:].rearrange("p (a b) -> p a b", a=2),
                                 ps_O4K[:, :, 0:IW],
                                 s_gl44[:].rearrange("p (a b) -> p a b", a=2))
            s_O4 = small.tile([4, NI], F32)
            for h in range(2):
                sl = slice(h * IW, (h + 1) * IW)
                ps = psS.tile([4, 512], F32, tag="t", name="ps_o4")
                nc.tensor.matmul(ps[:, 0:IW], s_cs[:], prod[:, sl],
                                 start=True, stop=True)
                nc.scalar.activation(s_O4[:, sl], ps[:, 0:IW], AF.Copy,
                                     bias=0.0)
            # broadcast the denominator row to partitions 0..2 (engines need
            # partition starts in {0,32,64,96}), then reciprocal + multiply.
            # Guard max(x, 0.1): valid rows have denom >= 2*e^-2, invalid
            # ones are overwritten by rv=0 below.
            s_rs3 = small.tile([3, NI], F32)
            for r in range(3):
                nc.sync.dma_start(s_rs3[r:r + 1, :], s_O4[3:4, :])
            s_rg = small.tile([3, NI], F32)
            nc.vector.tensor_scalar(s_rg[:], s_rs3[:], 0.1, None, ALU.max)
            s_rln = small.tile([3, NI], F32)
            nc.scalar.activation(s_rln[:], s_rg[:], AF.Ln, bias=b_zero[0:3, :])
            s_rcp = small.tile([3, NI], F32)
            nc.scalar.activation(s_rcp[:], s_rln[:], AF.Exp,
                                 bias=b_zero[0:3, :], scale=-1.0)
            s_res = small.tile([3, NI], F32)
            nc.vector.tensor_mul(s_res[:], s_O4[0:3, :], s_rcp[:])
            for h in range(2):
                sl = slice(h * IW, (h + 1) * IW)
                ps_rv = psS.tile([3, 512], F32, tag="t", name="ps_rv")
                nc.tensor.matmul(ps_rv[:, 0:IW], valid3[:], s_gls[:, sl],
                                 start=True, stop=True)
                # out = (Od + 1) * rowvalid - 1 (exact select for rv in {0,1})
                nc.vector.scalar_tensor_tensor(
                    s_res[:, sl], s_res[:, sl], 1.0,
                    ps_rv[:, 0:IW], ALU.add, ALU.mult)
            nc.scalar.add(s_res[:], s_res[:], b_neg1[0:3, :])
            nc.sync.dma_start(d_out[:], s_res[:])

    return nc


_NC_CACHE = None


def _get_nc():
    global _NC_CACHE
    if _NC_CACHE is None:
        _NC_CACHE = _build_nc()
    return _NC_CACHE


def build_in_maps(gray_feature, rgb_feature, rgb_image, gray_label, rgb_label):
    import ml_dtypes
    f8 = ml_dtypes.float8_e4m3

    gf_all = np.ascontiguousarray(gray_feature, np.float32).reshape(B, C, N)
    rf_all = np.ascontiguousarray(rgb_feature, np.float32).reshape(B, C, N)
    img_all = np.ascontiguousarray(rgb_image, np.float32).reshape(B, 3, N)
    gl_all = np.ascontiguousarray(gray_label, np.float32).reshape(B, NCH, N)
    rl_all = np.ascontiguousarray(rgb_label, np.float32).reshape(B, NCH, N)

    kc = np.concatenate([np.eye(NK, dtype=np.float16)] * 4, axis=1)
    cs = np.repeat(np.eye(4, dtype=np.float16), NK, axis=0)

    def to_T(chans):  # [X, N] -> [128, JC, X] pixel-transposed
        return np.ascontiguousarray(
            chans.T.reshape(JC, 128, -1).transpose(1, 0, 2))

    in_maps = []
    for core in range(NCORES):
        b, q = divmod(core, QS)
        sl = slice(q * NI, (q + 1) * NI)
        gf = gf_all[b]
        rf = rf_all[b]
        gl = gl_all[b][1:]
        rl = rl_all[b][1:]
        gf_aug = np.concatenate(
            [gf, np.ones((1, N), np.float32), np.zeros((1, N), np.float32)], 0)
        rf_aug = np.concatenate(
            [rf, np.ones((1, N), np.float32), np.zeros((1, N), np.float32)], 0)
        img4 = np.concatenate([img_all[b], np.ones((1, N), np.float32)], 0)
        i4r = (img4[:, None, :] * rl[None, :, :]).reshape(M4, N)
        in_maps.append({
            "glT8": to_T(gl).astype(f8),
            "gfT8": to_T(gf_aug).astype(f8),
            "rlT8": to_T(rl).astype(f8),
            "rfT8": to_T(rf_aug).astype(f8),
            "gls": np.ascontiguousarray(gl[:, sl]).astype(np.float16),
            "gfs": np.ascontiguousarray(
                gf[:, sl].reshape(CC, 128, NI).transpose(1, 0, 2)
            ).astype(np.float16),
            "rl": rl.astype(np.float16),
            "rf": np.ascontiguousarray(
                rf.reshape(CC, 128, N).transpose(1, 0, 2)).astype(np.float16),
            "i4r": to_T(i4r).astype(np.float16),
            "kc": kc,
            "cs": cs,
        })
    return in_maps


def kernel(gray_feature, rgb_feature, rgb_image, gray_label, rgb_label):
    in_maps = build_in_maps(gray_feature, rgb_feature, rgb_image,
                            gray_label, rgb_label)
    res = run_bass_kernel_spmd(_get_nc(), in_maps, list(range(NCORES)))

    canvas = np.empty((B, 3, N), np.float32)
    for core in range(NCORES):
        b, q = divmod(core, QS)
        canvas[b, :, q * NI:(q + 1) * NI] = res.results[core]["out"]
    return canvas.reshape(B, 3, H, W)


# revision 16
# speedup vs baseline: 2.0021x; 1.1111x over previous
"""Trainium2 Bass kernel for nn_C_Net_77807627534400 (sparse_attention).

Reference semantics: for each batch image and each class k in 1..11, the
per-class masked-normalized gray/rgb features form an [N,N] correlation,
softmax over the rgb-mask pixels, and a weighted mean of the rgb image is
written at the gray-mask pixels (if both masks have >= 2 pixels).

Because every pixel belongs to exactly one class, the 11 per-class [N,N]
matmuls fuse into ONE [N,N] matmul of per-class-centered features. The
class-match mask is enforced EXACTLY in the output matmul: expand img4
(rgb + ones row) to 44 rows IMG4R[(c,k), j] = img4[c,j] * rl[k,j], so

    O4K[(c,k), i] = sum_j img4[c,j] rl[k,j] e[j,i]
    O4[c, i]      = sum_k gl[k,i] O4K[(c,k), i]     (per-i class select)

with e[j,i] = exp(corr[j,i] - 1) computed WITHOUT any masking bias. The
collapse is an elementwise multiply by gl44 (gl broadcast to 44 rows via a
tiny matmul) plus two [44 -> 3] summing matmuls per half: one produces
numerator+denominator, the other the denominator replicated onto partitions
0..2 (avoids any cross-partition moves in the tail).

Normalization: gray side is explicitly normalized and scaled by 16 into
fp8 range (unit16 = 16 * bar / ||bar||); the rgb side is NOT normalized --
raw centered bar_r is the matmul operand and rsqrt(ssq_r)/16 is applied as
the per-partition *scale* of the Exp activation. Per-j sumsq is computed in
j-partition layout with tiny N=1 matmuls. All rsqrt/reciprocal come from
exp(a*ln(x) + b) so ScalarE only ever loads the natural_log_exp_and_others
table set (exp/ln/copy/square live there) -- exactly one ACT_TABLE_LOAD.

Dtypes: fp8e4 (e4m3) for every large matmul operand; the big attention and
class-means matmuls run DoubleRow (K packed 2x128, 0.5 cycles/row). PSUM is
fp32; the softmax weighted-average structure keeps fp8 quantization noise
(~6% per element, averaged over ~450 mask pixels) far below the 2e-2
tolerance. All DRAM tensors are host-side laid out to exactly match their
SBUF tiles so every DMA is contiguous.

Sharding: 8 cores = 2 batches x 4 slices of 576 gray pixels. Each core
computes the full rgb side for its batch (redundant across 4 cores) and its
576-column slice of the gray side.
"""

import numpy as np

import concourse.bass as bass
import concourse.tile as tile
from concourse import mybir
from concourse.bass_utils import run_bass_kernel_spmd
from concourse.vector_clock import ScopedClock

B, C, H, W, NCH = 2, 256, 48, 48, 12
N = H * W           # 2304
NK = NCH - 1        # classes 1..11
QS = 4              # gray-pixel slices per batch
NI = N // QS        # 576 rows per core
NCORES = B * QS     # 8
JC = N // 128       # 18 j-chunks
JP = JC // 2        # 9 j-chunk pairs
CC = C // 128       # 2 c-chunks
IW = 288            # i-chunk width (two per slice)
RW = 256            # rgb normalize chunk width
NRC = N // RW       # 9 rgb chunks
M4 = 4 * NK         # 44 expanded img rows
M4P = 48            # M4 padded so DoubleRow plane strides are 16B-aligned
NKP = 16            # NK padded likewise for the transposed labels
F272 = 272          # 258 feature+count cols padded likewise
LN16 = float(np.log(16.0))
F32 = mybir.dt.float32
F16 = mybir.dt.float16
F8 = mybir.dt.float8e4
ALU = mybir.AluOpType
AF = mybir.ActivationFunctionType
DR = mybir.MatmulPerfMode.DoubleRow


class _TC(tile.TileContext):
    """Workaround: this walrus build rejects instructions carrying more than
    one sync-wait command. Split every multi-wait instruction into a chain of
    single-wait NOPs (same engine, program order preserved) followed by the
    original instruction holding the final wait."""

    def _add_instruction(self, inst):
        si = inst.sync_info
        if si is not None:
            waits = list(si.on_wait)
            if len(waits) > 1:
                nc = self.nc
                for w in waits[:-1]:
                    nop = mybir.InstNoOp(
                        name=nc.get_next_instruction_name(),
                        sync_info=mybir.SyncInfo(on_wait=[w], on_update=[]),
                        bass_nofuse=True,
                        engine=inst.engine,
                    )
                    super()._add_instruction(nop)
                si.on_wait = waits[-1:]
                inst.sync_info = si
        super()._add_instruction(inst)

    def _drain_and_barrier(self, tick_clock, wait_clock):
        nc = self.nc
        drain_inst = nc.sync.drain()
        wait_clock.add_sem_waits(
            drain_inst.ins, ScopedClock({None: tick_clock.global_clock})
        )
        si = drain_inst.ins.sync_info
        waits = list(si.on_wait) if si is not None else []
        if len(waits) > 1:
            si.on_wait = waits[:1]
            drain_inst.ins.sync_info = si
            for w in waits[1:]:
                extra = nc.sync.drain()
                extra.ins.sync_info = mybir.SyncInfo(on_wait=[w], on_update=[])

        nc.all_engine_barrier()
        assert self.sems is not None
        popped = nc._tile_sem_poison_stack.pop()
        assert popped is self._sem_poison
        nc.clear_and_free_semaphores(list(self.sems.allocated().values()))
        nc.all_engine_barrier()


def _build_nc():
    nc = bass.Bass(target_bir_lowering=False)

    d_glT = nc.dram_tensor("glT8", [128, JP, 2, NKP], F8, kind="ExternalInput")
    d_gfT = nc.dram_tensor("gfT8", [128, JP, 2, F272], F8, kind="ExternalInput")
    d_rlT = nc.dram_tensor("rlT8", [128, JP, 2, NKP], F8, kind="ExternalInput")
    d_rfT = nc.dram_tensor("rfT8", [128, JP, 2, F272], F8, kind="ExternalInput")
    d_gls = nc.dram_tensor("gls", [NK, NI], F8, kind="ExternalInput")
    d_gfs = nc.dram_tensor("gfs", [128, CC, NI], F8, kind="ExternalInput")
    d_rl = nc.dram_tensor("rl", [NK, N], F8, kind="ExternalInput")
    d_rf = nc.dram_tensor("rf", [128, CC, N], F8, kind="ExternalInput")
    d_i4r = nc.dram_tensor("i4r", [128, JP, 2, M4P], F8, kind="ExternalInput")
    d_kc = nc.dram_tensor("kc", [NK, M4P], F8, kind="ExternalInput")
    d_csn = nc.dram_tensor("csn", [M4P, 3], F16, kind="ExternalInput")
    d_csd = nc.dram_tensor("csd", [M4P, 3], F16, kind="ExternalInput")
    d_out = nc.dram_tensor("out", [3, NI], F32, kind="ExternalOutput")

    with _TC(nc) as tc:
        with (
            tc.tile_pool(name="big", bufs=1) as big,
            tc.tile_pool(name="work", bufs=1) as work,
            tc.tile_pool(name="chk", bufs=2) as chk,
            tc.tile_pool(name="expp", bufs=3) as expp,
            tc.tile_pool(name="small", bufs=1) as small,
            tc.tile_pool(name="psS", bufs=2, space="PSUM") as psS,
            tc.tile_pool(name="psM", bufs=2, space="PSUM") as psM,
            tc.tile_pool(name="psO", bufs=1, space="PSUM") as psO,
        ):
            # ---- loads, in consumption order ----
            s_glT = big.tile([128, JP, 2, NKP], F8)
            nc.sync.dma_start(s_glT[:], d_glT[:])
            s_gfT = big.tile([128, JP, 2, F272], F8)
            for p in range(0, JP, 3):
                nc.sync.dma_start(s_gfT[:, p:p + 3, :, :], d_gfT[:, p:p + 3, :, :])
            s_rlT = big.tile([128, JP, 2, NKP], F8)
            nc.sync.dma_start(s_rlT[:], d_rlT[:])
            s_rfT = big.tile([128, JP, 2, F272], F8)
            for p in range(0, JP, 3):
                nc.sync.dma_start(s_rfT[:, p:p + 3, :, :], d_rfT[:, p:p + 3, :, :])
            s_gls = big.tile([NK, NI], F8)
            nc.sync.dma_start(s_gls[:], d_gls[:])
            s_gfs = big.tile([128, CC, NI], F8)
            nc.sync.dma_start(s_gfs[:], d_gfs[:])
            s_rl = big.tile([NK, N], F8)
            nc.sync.dma_start(s_rl[:], d_rl[:])
            s_rf = big.tile([128, CC, N], F8)
            for p in range(0, N, 1152):
                nc.sync.dma_start(s_rf[:, :, p:p + 1152], d_rf[:, :, p:p + 1152])
            s_i4r = big.tile([128, JP, 2, M4P], F8)
            nc.sync.dma_start(s_i4r[:], d_i4r[:])
            s_kc = big.tile([NK, M4P], F8)
            nc.sync.dma_start(s_kc[:], d_kc[:])
            s_csn = big.tile([M4P, 3], F16)
            nc.sync.dma_start(s_csn[:], d_csn[:])
            s_csd = big.tile([M4P, 3], F16)
            nc.sync.dma_start(s_csd[:], d_csd[:])

            # on-chip constants
            s_ones16 = big.tile([128, 128], F16)
            nc.vector.memset(s_ones16[:], 1.0)
            b_zero = big.tile([128, 1], F32)
            nc.vector.memset(b_zero[:], 0.0)
            b_eps = big.tile([128, 1], F32)
            nc.vector.memset(b_eps[:], 1e-4)
            b_neg1 = big.tile([128, 1], F32)
            nc.vector.memset(b_neg1[:], -1.0)
            b_pln16 = big.tile([128, 1], F32)
            nc.vector.memset(b_pln16[:], LN16)
            b_nln16 = big.tile([128, 1], F32)
            nc.vector.memset(b_nln16[:], -LN16)

            # ---- per-class sums + counts (col 256 is the ones column) ----
            def class_means(s_lT, s_fT, nmtag):
                ps = psS.tile([NKP, 512], F32, tag="t", name=f"ps_mean{nmtag}")
                for p in range(JP):
                    nc.tensor.matmul(ps[:, 0:F272], s_lT[:, p, :, :],
                                     s_fT[:, p, :, :], perf_mode=DR,
                                     start=(p == 0), stop=(p == JP - 1))
                cnt = small.tile([NK, 1], F32, name=f"cnt{nmtag}")
                nc.any.tensor_copy(cnt[:], ps[0:NK, 256:257])
                rc = small.tile([NK, 1], F32, name=f"rc{nmtag}")
                nc.vector.tensor_scalar(rc[:], cnt[:], 1.0, None, ALU.max)
                nc.vector.reciprocal(rc[:], rc[:])
                meanT = work.tile([NK, C], F8, name=f"mean{nmtag}")
                nc.vector.tensor_scalar(meanT[:], ps[0:NK, 0:C], rc[:], None,
                                        ALU.mult)
                return meanT, cnt

            meanT_g, cnt_g = class_means(s_glT, s_gfT, "g")
            meanT_r, cnt_r = class_means(s_rlT, s_rfT, "r")
            vg = small.tile([NK, 1], F32)
            nc.vector.tensor_scalar(vg[:], cnt_g[:], 1.5, None, ALU.is_gt)
            valid = small.tile([NK, 1], F32)
            nc.vector.tensor_scalar(valid[:], cnt_r[:], 1.5, None, ALU.is_gt)
            nc.vector.tensor_mul(valid[:], valid[:], vg[:])
            valid3 = small.tile([NK, 3], F8)
            for i in range(3):
                nc.any.tensor_copy(valid3[:, i:i + 1], valid[:])

            # gl44[(c,k), i] = gl[k, i] for the final class-collapse
            s_gl44 = small.tile([M4P, NI], F16)
            for h in range(2):
                sl = slice(h * IW, (h + 1) * IW)
                ps = psS.tile([M4P, 512], F32, tag="t", name="ps_gl44")
                nc.tensor.matmul(ps[:, 0:IW], s_kc[:], s_gls[:, sl],
                                 start=True, stop=True)
                nc.any.tensor_copy(s_gl44[:, sl], ps[:, 0:IW])

            # ---- gray side: unit16_g = 16 * (gf - mu) / ||gf - mu|| ----
            unit_g = work.tile([128, CC, NI], F8, name="unitg")
            for ib in range(2):
                sl = slice(ib * IW, (ib + 1) * IW)
                barg = [chk.tile([128, IW], F16, tag=f"barg{cc}", bufs=2,
                                 name=f"barg{cc}") for cc in range(CC)]
                sqg = [chk.tile([128, IW], F16, tag=f"sqg{cc}", bufs=2,
                                name=f"sqg{cc}") for cc in range(CC)]
                for cc in range(CC):
                    ps = psS.tile([128, 512], F32, tag="t", name="ps_mug")
                    nc.tensor.matmul(ps[:, 0:IW],
                                     meanT_g[:, cc * 128:(cc + 1) * 128],
                                     s_gls[:, sl], start=True, stop=True)
                    nc.any.tensor_sub(barg[cc][:], s_gfs[:, cc, sl],
                                      ps[:, 0:IW])
                    nc.any.tensor_mul(sqg[cc][:], barg[cc][:], barg[cc][:])
                ps2 = psS.tile([128, 512], F32, tag="t", name="ps_ssqg")
                for cc in range(CC):
                    nc.tensor.matmul(ps2[:, 0:IW], s_ones16[:], sqg[cc][:],
                                     start=(cc == 0), stop=(cc == CC - 1))
                lng = chk.tile([128, IW], F32, tag="lng", bufs=2, name="lng")
                nc.scalar.activation(lng[:], ps2[:, 0:IW], AF.Ln,
                                     bias=b_eps[:])
                rbg = chk.tile([128, IW], F32, tag="rbg", bufs=2, name="rbg")
                nc.scalar.activation(rbg[:], lng[:], AF.Exp,
                                     bias=b_pln16[:], scale=-0.5)
                for cc in range(CC):
                    nc.any.tensor_mul(unit_g[:, cc, sl], barg[cc][:], rbg[:])

            # ---- rgb side: bar_r chunks (fp8, DoubleRow layout) + per-j
            # sumsq in j-partition layout; rsqrt/16 becomes the Exp scale ----
            bar_r = {}
            # rsqrt batches: A = chunks 0-1 (jc 0-3), B = chunks 2-4
            # (jc 4-9), C = chunks 5-8 (jc 10-17)
            ssq = [small.tile([128, 4], F32, name="ssqA"),
                   small.tile([128, 6], F32, name="ssqB"),
                   small.tile([128, 8], F32, name="ssqC")]
            rsq = [small.tile([128, 4], F32, name="rsqA"),
                   small.tile([128, 6], F32, name="rsqB"),
                   small.tile([128, 8], F32, name="rsqC")]
            BASE = [0, 4, 10]

            def batch_of(jc):
                bi = 0 if jc < 4 else (1 if jc < 10 else 2)
                return bi, jc - BASE[bi]

            def r_chunk(ib):
                sl = slice(ib * RW, (ib + 1) * RW)
                bar8 = chk.tile([128, 2, RW], F8, tag="bar8", bufs=10,
                                name="bar8")
                sq8 = chk.tile([128, 2, RW], F16, tag="sq8", bufs=3,
                               name="sq8")
                ps = psS.tile([128, 2, RW], F32, tag="t", name="ps_mur")
                for cc in range(CC):
                    nc.tensor.matmul(ps[:, cc, :],
                                     meanT_r[:, cc * 128:(cc + 1) * 128],
                                     s_rl[:, sl], start=True, stop=True)
                nc.any.tensor_sub(bar8[:], s_rf[:, :, sl], ps[:, :, :])
                nc.any.tensor_mul(sq8[:], bar8[:], bar8[:])
                for h in range(2):
                    jc = 2 * ib + h
                    lo = h * 128
                    ps2 = psS.tile([128, 512], F32, tag="t", name="ps_ssqr")
                    for cc in range(CC):
                        nc.tensor.matmul(ps2[:, 0:1], sq8[:, cc, lo:lo + 128],
                                         s_ones16[:, 0:1],
                                         start=(cc == 0), stop=(cc == CC - 1))
                    bi, col = batch_of(jc)
                    nc.any.tensor_copy(ssq[bi][:, col:col + 1], ps2[:, 0:1])
                bar_r[ib] = bar8

            def rsqrt_batch(bi):
                w = ssq[bi].shape[1]
                t = small.tile([128, 8], F32, name=f"lnr{bi}")
                nc.scalar.activation(t[:, 0:w], ssq[bi][:], AF.Ln,
                                     bias=b_eps[:])
                nc.scalar.activation(rsq[bi][:], t[:, 0:w], AF.Exp,
                                     bias=b_nln16[:], scale=-0.5)

            # ---- attention pairs + masked-output accumulation ----
            ps_O4K = psO.tile([M4P, 2, 512], F32)

            def attention_pair(pr):
                s_exp = expp.tile([128, 2, NI], F8, tag="exp", name="s_exp")
                for h in range(2):
                    jc = 2 * pr + h
                    ib, lo = jc // 2, (jc % 2) * 128
                    bar8 = bar_r[ib]
                    ps_mt = psM.tile([128, 2, 512], F32, tag="mt",
                                     name="ps_mt")
                    for ic in range(2):
                        i0 = ic * IW
                        nc.tensor.matmul(ps_mt[:, ic, 0:IW],
                                         bar8[:, :, lo:lo + 128],
                                         unit_g[:, :, i0:i0 + IW],
                                         perf_mode=DR, start=True, stop=True)
                    bi, col = batch_of(jc)
                    nc.scalar.activation(
                        s_exp[:, h, :].rearrange("p (a b) -> p a b", a=2),
                        ps_mt[:, :, 0:IW], AF.Exp, bias=b_neg1[:],
                        scale=rsq[bi][:, col:col + 1])
                for ic in range(2):
                    i0 = ic * IW
                    nc.tensor.matmul(ps_O4K[:, ic, 0:IW], s_i4r[:, pr, :, :],
                                     s_exp[:, :, i0:i0 + IW], perf_mode=DR,
                                     start=(pr == 0), stop=(pr == JP - 1))

            # schedule: chunks 0-1 up front unlock pairs 0-1 (jc 0-3);
            # chunks 2-4 + batch B are emitted during pairs 0-1 (B is read
            # from pair 2 = jc 4); chunks 5-8 + batch C during pairs 2-4
            # (C is read from pair 5 = jc 10)
            r_chunk(0)
            r_chunk(1)
            rsqrt_batch(0)
            NEXT = {0: [2, 3], 1: [4], 2: [5], 3: [6], 4: [7, 8]}
            for pr in range(JP):
                attention_pair(pr)
                for nxt in NEXT.get(pr, []):
                    r_chunk(nxt)
                    if nxt == 4:
                        rsqrt_batch(1)
                    if nxt == 8:
                        rsqrt_batch(2)

            # ---- finalize: class-collapse, divide by row-sum, validity ----
            # csn collapses to numerator+denominator rows 0..2; csd
            # replicates the denominator onto rows 0..2 directly.
            prod = small.tile([M4P, NI], F16)
            nc.any.tensor_mul(prod[:].rearrange("p (a b) -> p a b", a=2),
                              ps_O4K[:, :, 0:IW],
                              s_gl44[:].rearrange("p (a b) -> p a b", a=2))
            s_res = small.tile([3, NI], F32)
            s_rg = small.tile([3, NI], F32)
            s_rln = small.tile([3, NI], F32)
            s_rcp = small.tile([3, NI], F32)
            for h in range(2):
                sl = slice(h * IW, (h + 1) * IW)
                ps_nd = psS.tile([3, 512], F32, tag="t", name="ps_nd")
                nc.tensor.matmul(ps_nd[:, 0:IW], s_csn[:], prod[:, sl],
                                 start=True, stop=True)
                ps_dn = psS.tile([3, 512], F32, tag="t", name="ps_dn")
                nc.tensor.matmul(ps_dn[:, 0:IW], s_csd[:], prod[:, sl],
                                 start=True, stop=True)
                # rcp = 1/max(den, 0.1): valid rows have den >= 2*e^-2,
                # invalid ones are zeroed by rv below
                nc.any.tensor_scalar(s_rg[:, sl], ps_dn[:, 0:IW], 0.1, None,
                                     ALU.max)
                nc.scalar.activation(s_rln[:, sl], s_rg[:, sl], AF.Ln,
                                     bias=b_zero[0:3, :])
                nc.scalar.activation(s_rcp[:, sl], s_rln[:, sl], AF.Exp,
                                     bias=b_zero[0:3, :], scale=-1.0)
                # (num+den)/den = out+1; then multiply by validity and -1
                nc.any.tensor_mul(s_res[:, sl], ps_nd[:, 0:IW],
                                  s_rcp[:, sl])
                ps_rv = psS.tile([3, 512], F32, tag="t", name="ps_rv")
                nc.tensor.matmul(ps_rv[:, 0:IW], valid3[:], s_gls[:, sl],
                                 start=True, stop=True)
                nc.any.tensor_mul(s_res[:, sl], s_res[:, sl], ps_rv[:, 0:IW])
                nc.any.tensor_scalar(s_res[:, sl], s_res[:, sl], -1.0, None,
                                     ALU.add)
            nc.sync.dma_start(d_out[:], s_res[:])

    return nc


_NC_CACHE = None


def _get_nc():
    global _NC_CACHE
    if _NC_CACHE is None:
        _NC_CACHE = _build_nc()
    return _NC_CACHE


def build_in_maps(gray_feature, rgb_feature, rgb_image, gray_label, rgb_label):
    import ml_dtypes
    f8 = ml_dtypes.float8_e4m3

    gf_all = np.ascontiguousarray(gray_feature, np.float32).reshape(B, C, N)
    rf_all = np.ascontiguousarray(rgb_feature, np.float32).reshape(B, C, N)
    img_all = np.ascontiguousarray(rgb_image, np.float32).reshape(B, 3, N)
    gl_all = np.ascontiguousarray(gray_label, np.float32).reshape(B, NCH, N)
    rl_all = np.ascontiguousarray(rgb_label, np.float32).reshape(B, NCH, N)

    kc = np.zeros((NK, M4P), np.float32)
    kc[:, 0:M4] = np.concatenate([np.eye(NK, dtype=np.float32)] * 4, axis=1)
    # csn: (c == c2) + (c == 3) -> numerator + denominator; csd: (c == 3)
    cblk = np.repeat(np.arange(4), NK)
    csn = np.zeros((M4P, 3), np.float16)
    csd = np.zeros((M4P, 3), np.float16)
    for c2 in range(3):
        csn[0:M4, c2] = (cblk == c2) + (cblk == 3)
        csd[0:M4, c2] = (cblk == 3)

    def to_T(chans, xp=None):
        # [X, N] -> [128, JP, 2, XP] pixel-transposed, DoubleRow-packed,
        # zero-padded along the last dim to a 16-byte multiple
        x = chans.shape[0]
        arr = chans.T.reshape(JP, 2, 128, x).transpose(2, 0, 1, 3)
        if xp is not None and xp != x:
            out = np.zeros((128, JP, 2, xp), arr.dtype)
            out[..., 0:x] = arr
            arr = out
        return np.ascontiguousarray(arr)

    in_maps = []
    for core in range(NCORES):
        b, q = divmod(core, QS)
        sl = slice(q * NI, (q + 1) * NI)
        gf = gf_all[b]
        rf = rf_all[b]
        gl = gl_all[b][1:]
        rl = rl_all[b][1:]
        gf_aug = np.concatenate(
            [gf, np.ones((1, N), np.float32), np.zeros((1, N), np.float32)], 0)
        rf_aug = np.concatenate(
            [rf, np.ones((1, N), np.float32), np.zeros((1, N), np.float32)], 0)
        img4 = np.concatenate([img_all[b], np.ones((1, N), np.float32)], 0)
        i4r = (img4[:, None, :] * rl[None, :, :]).reshape(M4, N)
        in_maps.append({
            "glT8": to_T(gl, NKP).astype(f8),
            "gfT8": to_T(gf_aug, F272).astype(f8),
            "rlT8": to_T(rl, NKP).astype(f8),
            "rfT8": to_T(rf_aug, F272).astype(f8),
            "gls": np.ascontiguousarray(gl[:, sl]).astype(f8),
            "gfs": np.ascontiguousarray(
                gf[:, sl].reshape(CC, 128, NI).transpose(1, 0, 2)).astype(f8),
            "rl": rl.astype(f8),
            "rf": np.ascontiguousarray(
                rf.reshape(CC, 128, N).transpose(1, 0, 2)).astype(f8),
            "i4r": to_T(i4r, M4P).astype(f8),
            "kc": kc.astype(f8),
            "csn": csn,
            "csd": csd,
        })
    return in_maps


def kernel(gray_feature, rgb_feature, rgb_image, gray_label, rgb_label):
    in_maps = build_in_maps(gray_feature, rgb_feature, rgb_image,
                            gray_label, rgb_label)
    res = run_bass_kernel_spmd(_get_nc(), in_maps, list(range(NCORES)))

    canvas = np.empty((B, 3, N), np.float32)
    for core in range(NCORES):
        b, q = divmod(core, QS)
        canvas[b, :, q * NI:(q + 1) * NI] = res.results[core]["out"]
    return canvas.reshape(B, 3, H, W)


# revision 21
# speedup vs baseline: 2.0410x; 1.0194x over previous
"""Trainium2 Bass kernel for nn_C_Net_77807627534400 (sparse_attention).

Reference semantics: for each batch image and each class k in 1..11, the
per-class masked-normalized gray/rgb features form an [N,N] correlation,
softmax over the rgb-mask pixels, and a weighted mean of the rgb image is
written at the gray-mask pixels (if both masks have >= 2 pixels).

Because every pixel belongs to exactly one class, the 11 per-class [N,N]
matmuls fuse into ONE [N,N] matmul of per-class-centered features. The
class-match mask is enforced EXACTLY in the output matmul: expand img4
(rgb + ones row) to 44 rows IMG4R[(c,k), j] = img4[c,j] * rl[k,j], so

    O4K[(c,k), i] = sum_j img4[c,j] rl[k,j] e[j,i]
    O4[c, i]      = sum_k gl[k,i] O4K[(c,k), i]     (per-i class select)

with e[j,i] = exp(corr[j,i] - 1) computed WITHOUT any masking bias. The
collapse is an elementwise multiply by gl44 (gl broadcast to 44 rows via a
tiny matmul) plus two [44 -> 3] summing matmuls per half: one produces
numerator+denominator, the other the denominator replicated onto partitions
0..2 (avoids any cross-partition moves in the tail).

Normalization: gray side is explicitly normalized and scaled by 16 into
fp8 range (unit16 = 16 * bar / ||bar||); the rgb side is NOT normalized --
raw centered bar_r is the matmul operand and rsqrt(ssq_r)/16 is applied as
the per-partition *scale* of the Exp activation. Per-j sumsq is computed in
j-partition layout with tiny N=1 matmuls. All rsqrt/reciprocal come from
exp(a*ln(x) + b) so ScalarE only ever loads the natural_log_exp_and_others
table set (exp/ln/copy/square live there) -- exactly one ACT_TABLE_LOAD.

Dtypes: fp8e4 (e4m3) for every large matmul operand; the big attention and
class-means matmuls run DoubleRow (K packed 2x128, 0.5 cycles/row). PSUM is
fp32; the softmax weighted-average structure keeps fp8 quantization noise
(~6% per element, averaged over ~450 mask pixels) far below the 2e-2
tolerance. All DRAM tensors are host-side laid out to exactly match their
SBUF tiles so every DMA is contiguous.

Sharding: 8 cores = 2 batches x 4 slices of 576 gray pixels. Each core
computes the full rgb side for its batch (redundant across 4 cores) and its
576-column slice of the gray side.
"""

import numpy as np

import concourse.bass as bass
import concourse.tile as tile
from concourse import mybir
from concourse.bass_utils import run_bass_kernel_spmd
from concourse.vector_clock import ScopedClock

B, C, H, W, NCH = 2, 256, 48, 48, 12
N = H * W           # 2304
NK = NCH - 1        # classes 1..11
QS = 4              # gray-pixel slices per batch
NI = N // QS        # 576 rows per core
NCORES = B * QS     # 8
JC = N // 128       # 18 j-chunks
JP = JC // 2        # 9 j-chunk pairs
CC = C // 128       # 2 c-chunks
IW = 288            # i-chunk width (two per slice)
RW = 256            # rgb normalize chunk width
NRC = N // RW       # 9 rgb chunks
M4 = 4 * NK         # 44 expanded img rows
M4P = 48            # M4 padded so DoubleRow plane strides are 16B-aligned
NKP = 16            # NK padded likewise for the transposed labels
F272 = 272          # 258 feature+count cols padded likewise
LN16 = float(np.log(16.0))
F32 = mybir.dt.float32
F16 = mybir.dt.float16
F8 = mybir.dt.float8e4
ALU = mybir.AluOpType
AF = mybir.ActivationFunctionType
DR = mybir.MatmulPerfMode.DoubleRow


class _TC(tile.TileContext):
    """Workaround: this walrus build rejects instructions carrying more than
    one sync-wait command. Split every multi-wait instruction into a chain of
    single-wait NOPs (same engine, program order preserved) followed by the
    original instruction holding the final wait."""

    def _add_instruction(self, inst):
        si = inst.sync_info
        if si is not None:
            waits = list(si.on_wait)
            if len(waits) > 1:
                nc = self.nc
                for w in waits[:-1]:
                    nop = mybir.InstNoOp(
                        name=nc.get_next_instruction_name(),
                        sync_info=mybir.SyncInfo(on_wait=[w], on_update=[]),
                        bass_nofuse=True,
                        engine=inst.engine,
                    )
                    super()._add_instruction(nop)
                si.on_wait = waits[-1:]
                inst.sync_info = si
        super()._add_instruction(inst)

    def _drain_and_barrier(self, tick_clock, wait_clock):
        nc = self.nc
        drain_inst = nc.sync.drain()
        wait_clock.add_sem_waits(
            drain_inst.ins, ScopedClock({None: tick_clock.global_clock})
        )
        si = drain_inst.ins.sync_info
        waits = list(si.on_wait) if si is not None else []
        if len(waits) > 1:
            si.on_wait = waits[:1]
            drain_inst.ins.sync_info = si
            for w in waits[1:]:
                extra = nc.sync.drain()
                extra.ins.sync_info = mybir.SyncInfo(on_wait=[w], on_update=[])

        nc.all_engine_barrier()
        assert self.sems is not None
        popped = nc._tile_sem_poison_stack.pop()
        assert popped is self._sem_poison
        nc.clear_and_free_semaphores(list(self.sems.allocated().values()))
        nc.all_engine_barrier()


def _build_nc():
    nc = bass.Bass(target_bir_lowering=False)

    d_glT = nc.dram_tensor("glT8", [128, JP, 2, NKP], F8, kind="ExternalInput")
    d_gfT = nc.dram_tensor("gfT8", [128, JP, 2, F272], F8, kind="ExternalInput")
    d_rlT = nc.dram_tensor("rlT8", [128, JP, 2, NKP], F8, kind="ExternalInput")
    d_rfT = nc.dram_tensor("rfT8", [128, JP, 2, F272], F8, kind="ExternalInput")
    d_gls = nc.dram_tensor("gls", [NK, NI], F8, kind="ExternalInput")
    d_gfs = nc.dram_tensor("gfs", [128, CC, NI], F8, kind="ExternalInput")
    d_rl = nc.dram_tensor("rl", [NK, N], F8, kind="ExternalInput")
    d_rf = nc.dram_tensor("rf", [128, CC, N], F8, kind="ExternalInput")
    d_i4r = nc.dram_tensor("i4r", [128, JP, 2, M4P], F8, kind="ExternalInput")
    d_kc = nc.dram_tensor("kc", [NK, M4P], F8, kind="ExternalInput")
    d_csn = nc.dram_tensor("csn", [M4P, 3], F16, kind="ExternalInput")
    d_csd = nc.dram_tensor("csd", [M4P, 3], F16, kind="ExternalInput")
    d_out = nc.dram_tensor("out", [3, NI], F32, kind="ExternalOutput")

    with _TC(nc) as tc:
        with (
            tc.tile_pool(name="big", bufs=1) as big,
            tc.tile_pool(name="work", bufs=1) as work,
            tc.tile_pool(name="chk", bufs=2) as chk,
            tc.tile_pool(name="expp", bufs=3) as expp,
            tc.tile_pool(name="small", bufs=1) as small,
            tc.tile_pool(name="psS", bufs=2, space="PSUM") as psS,
            tc.tile_pool(name="psM", bufs=2, space="PSUM") as psM,
            tc.tile_pool(name="psO", bufs=1, space="PSUM") as psO,
        ):
            # ---- loads, in consumption order ----
            s_glT = big.tile([128, JP, 2, NKP], F8)
            nc.sync.dma_start(s_glT[:], d_glT[:])
            s_gfT = big.tile([128, JP, 2, F272], F8)
            for p in range(0, JP, 3):
                nc.sync.dma_start(s_gfT[:, p:p + 3, :, :], d_gfT[:, p:p + 3, :, :])
            s_rlT = big.tile([128, JP, 2, NKP], F8)
            nc.sync.dma_start(s_rlT[:], d_rlT[:])
            s_rfT = big.tile([128, JP, 2, F272], F8)
            for p in range(0, JP, 3):
                nc.sync.dma_start(s_rfT[:, p:p + 3, :, :], d_rfT[:, p:p + 3, :, :])
            s_gls = big.tile([NK, NI], F8)
            nc.sync.dma_start(s_gls[:], d_gls[:])
            s_kc = big.tile([NK, M4P], F8)
            nc.sync.dma_start(s_kc[:], d_kc[:])
            s_gfs = big.tile([128, CC, NI], F8)
            nc.sync.dma_start(s_gfs[:], d_gfs[:])
            s_rl = big.tile([NK, N], F8)
            nc.sync.dma_start(s_rl[:], d_rl[:])
            s_rf = big.tile([128, CC, N], F8)
            for p in range(0, N, 1152):
                nc.sync.dma_start(s_rf[:, :, p:p + 1152],
                                  d_rf[:, :, p:p + 1152])
            s_i4r = big.tile([128, JP, 2, M4P], F8)
            nc.sync.dma_start(s_i4r[:], d_i4r[:])
            s_csn = big.tile([M4P, 3], F16)
            nc.sync.dma_start(s_csn[:], d_csn[:])
            s_csd = big.tile([M4P, 3], F16)
            nc.sync.dma_start(s_csd[:], d_csd[:])

            # on-chip constants
            s_ones16 = big.tile([128, 128], F16)
            nc.vector.memset(s_ones16[:], 1.0)
            b_zero = big.tile([128, 1], F32)
            nc.vector.memset(b_zero[:], 0.0)
            b_eps = big.tile([128, 1], F32)
            nc.vector.memset(b_eps[:], 1e-4)
            b_neg1 = big.tile([128, 1], F32)
            nc.vector.memset(b_neg1[:], -1.0)
            b_pln16 = big.tile([128, 1], F32)
            nc.vector.memset(b_pln16[:], LN16)
            b_nln16 = big.tile([128, 1], F32)
            nc.vector.memset(b_nln16[:], -LN16)

            # ---- per-class sums + counts (col 256 is the ones column) ----
            def class_means(s_lT, s_fT, nmtag):
                ps = psS.tile([NKP, 512], F32, tag="t", name=f"ps_mean{nmtag}")
                for p in range(JP):
                    nc.tensor.matmul(ps[:, 0:F272], s_lT[:, p, :, :],
                                     s_fT[:, p, :, :], perf_mode=DR,
                                     start=(p == 0), stop=(p == JP - 1))
                cnt = small.tile([NK, 1], F32, name=f"cnt{nmtag}")
                nc.any.tensor_copy(cnt[:], ps[0:NK, 256:257])
                rc = small.tile([NK, 1], F32, name=f"rc{nmtag}")
                nc.vector.tensor_scalar(rc[:], cnt[:], 1.0, None, ALU.max)
                nc.vector.reciprocal(rc[:], rc[:])
                meanT = work.tile([NK, C], F8, name=f"mean{nmtag}")
                nc.vector.tensor_scalar(meanT[:], ps[0:NK, 0:C], rc[:], None,
                                        ALU.mult)
                return meanT, cnt

            meanT_g, cnt_g = class_means(s_glT, s_gfT, "g")
            meanT_r, cnt_r = class_means(s_rlT, s_rfT, "r")
            vg = small.tile([NK, 1], F32)
            nc.vector.tensor_scalar(vg[:], cnt_g[:], 1.5, None, ALU.is_gt)
            valid = small.tile([NK, 1], F32)
            nc.vector.tensor_scalar(valid[:], cnt_r[:], 1.5, None, ALU.is_gt)
            nc.vector.tensor_mul(valid[:], valid[:], vg[:])
            valid3 = small.tile([NK, 3], F8)
            for i in range(3):
                nc.any.tensor_copy(valid3[:, i:i + 1], valid[:])

            # validity rows rv[c, i] = valid[k(i)] in SBUF, off the tail
            s_rvf = small.tile([3, NI], F32)
            for h in range(2):
                sl = slice(h * IW, (h + 1) * IW)
                ps = psS.tile([3, 512], F32, tag="t", name="ps_rv")
                nc.tensor.matmul(ps[:, 0:IW], valid3[:], s_gls[:, sl],
                                 start=True, stop=True)
                nc.any.tensor_copy(s_rvf[:, sl], ps[:, 0:IW])

            # gl44[(c,k), i] = gl[k, i] for the final class-collapse
            s_gl44 = small.tile([M4P, NI], F16)
            for h in range(2):
                sl = slice(h * IW, (h + 1) * IW)
                ps = psS.tile([M4P, 512], F32, tag="t", name="ps_gl44")
                nc.tensor.matmul(ps[:, 0:IW], s_kc[:], s_gls[:, sl],
                                 start=True, stop=True)
                nc.any.tensor_copy(s_gl44[:, sl], ps[:, 0:IW])

            # ---- gray side: unit16_g = 16 * (gf - mu) / ||gf - mu|| ----
            unit_g = work.tile([128, CC, NI], F8, name="unitg")
            for ib in range(2):
                sl = slice(ib * IW, (ib + 1) * IW)
                barg = [chk.tile([128, IW], F16, tag=f"barg{cc}", bufs=2,
                                 name=f"barg{cc}") for cc in range(CC)]
                sqg = [chk.tile([128, IW], F16, tag=f"sqg{cc}", bufs=2,
                                name=f"sqg{cc}") for cc in range(CC)]
                for cc in range(CC):
                    ps = psS.tile([128, 512], F32, tag="t", name="ps_mug")
                    nc.tensor.matmul(ps[:, 0:IW],
                                     meanT_g[:, cc * 128:(cc + 1) * 128],
                                     s_gls[:, sl], start=True, stop=True)
                    nc.any.tensor_sub(barg[cc][:], s_gfs[:, cc, sl],
                                      ps[:, 0:IW])
                    nc.any.tensor_mul(sqg[cc][:], barg[cc][:], barg[cc][:])
                ps2 = psS.tile([128, 512], F32, tag="t", name="ps_ssqg")
                for cc in range(CC):
                    nc.tensor.matmul(ps2[:, 0:IW], s_ones16[:], sqg[cc][:],
                                     start=(cc == 0), stop=(cc == CC - 1))
                lng = chk.tile([128, IW], F32, tag="lng", bufs=2, name="lng")
                nc.scalar.activation(lng[:], ps2[:, 0:IW], AF.Ln,
                                     bias=b_eps[:])
                rbg = chk.tile([128, IW], F32, tag="rbg", bufs=2, name="rbg")
                nc.scalar.activation(rbg[:], lng[:], AF.Exp,
                                     bias=b_pln16[:], scale=-0.5)
                for cc in range(CC):
                    nc.any.tensor_mul(unit_g[:, cc, sl], barg[cc][:], rbg[:])

            # ---- rgb side: bar_r chunks (fp8, DoubleRow layout) + per-j
            # sumsq in j-partition layout; rsqrt/16 becomes the Exp scale ----
            bar_r = {}
            # rsqrt batches: A = chunks 0-1 (jc 0-3), B = chunks 2-4
            # (jc 4-9), C = chunks 5-8 (jc 10-17)
            ssq = [small.tile([128, 4], F32, name="ssqA"),
                   small.tile([128, 6], F32, name="ssqB"),
                   small.tile([128, 8], F32, name="ssqC")]
            rsq = [small.tile([128, 4], F32, name="rsqA"),
                   small.tile([128, 6], F32, name="rsqB"),
                   small.tile([128, 8], F32, name="rsqC")]
            BASE = [0, 4, 10]

            def batch_of(jc):
                bi = 0 if jc < 4 else (1 if jc < 10 else 2)
                return bi, jc - BASE[bi]

            def r_chunk(ib):
                sl = slice(ib * RW, (ib + 1) * RW)
                bar8 = chk.tile([128, 2, RW], F8, tag="bar8", bufs=10,
                                name="bar8")
                sq8 = chk.tile([128, 2, RW], F16, tag="sq8", bufs=3,
                               name="sq8")
                ps = psS.tile([128, 2, RW], F32, tag="t", name="ps_mur")
                for cc in range(CC):
                    nc.tensor.matmul(ps[:, cc, :],
                                     meanT_r[:, cc * 128:(cc + 1) * 128],
                                     s_rl[:, sl], start=True, stop=True)
                nc.any.tensor_sub(bar8[:], s_rf[:, :, sl], ps[:, :, :])
                nc.any.tensor_mul(sq8[:], bar8[:], bar8[:])
                for h in range(2):
                    jc = 2 * ib + h
                    lo = h * 128
                    ps2 = psS.tile([128, 512], F32, tag="t", name="ps_ssqr")
                    for cc in range(CC):
                        nc.tensor.matmul(ps2[:, 0:1], sq8[:, cc, lo:lo + 128],
                                         s_ones16[:, 0:1],
                                         start=(cc == 0), stop=(cc == CC - 1))
                    bi, col = batch_of(jc)
                    nc.any.tensor_copy(ssq[bi][:, col:col + 1], ps2[:, 0:1])
                bar_r[ib] = bar8

            def rsqrt_batch(bi):
                w = ssq[bi].shape[1]
                t = small.tile([128, 8], F32, name=f"lnr{bi}")
                nc.scalar.activation(t[:, 0:w], ssq[bi][:], AF.Ln,
                                     bias=b_eps[:])
                nc.scalar.activation(rsq[bi][:], t[:, 0:w], AF.Exp,
                                     bias=b_nln16[:], scale=-0.5)

            # ---- attention pairs + masked-output accumulation ----
            ps_O4K = psO.tile([M4P, 2, 512], F32)

            def attention_pair(pr):
                s_exp = expp.tile([128, 2, NI], F8, tag="exp", name="s_exp")
                for h in range(2):
                    jc = 2 * pr + h
                    ib, lo = jc // 2, (jc % 2) * 128
                    bar8 = bar_r[ib]
                    ps_mt = psM.tile([128, 2, 512], F32, tag="mt",
                                     name="ps_mt")
                    for ic in range(2):
                        i0 = ic * IW
                        nc.tensor.matmul(ps_mt[:, ic, 0:IW],
                                         bar8[:, :, lo:lo + 128],
                                         unit_g[:, :, i0:i0 + IW],
                                         perf_mode=DR, start=True, stop=True)
                    bi, col = batch_of(jc)
                    nc.scalar.activation(
                        s_exp[:, h, :].rearrange("p (a b) -> p a b", a=2),
                        ps_mt[:, :, 0:IW], AF.Exp, bias=b_neg1[:],
                        scale=rsq[bi][:, col:col + 1])
                for ic in range(2):
                    i0 = ic * IW
                    nc.tensor.matmul(ps_O4K[:, ic, 0:IW], s_i4r[:, pr, :, :],
                                     s_exp[:, :, i0:i0 + IW], perf_mode=DR,
                                     start=(pr == 0), stop=(pr == JP - 1))

            # schedule: chunks 0-1 up front unlock pairs 0-1 (jc 0-3);
            # chunks 2-4 + batch B are emitted during pairs 0-1 (B is read
            # from pair 2 = jc 4); chunks 5-8 + batch C during pairs 2-4
            # (C is read from pair 5 = jc 10)
            r_chunk(0)
            r_chunk(1)
            rsqrt_batch(0)
            NEXT = {0: [2, 3], 1: [4], 2: [5], 3: [6], 4: [7, 8]}
            for pr in range(JP):
                attention_pair(pr)
                for nxt in NEXT.get(pr, []):
                    r_chunk(nxt)
                    if nxt == 4:
                        rsqrt_batch(1)
                    if nxt == 8:
                        rsqrt_batch(2)

            # ---- finalize: class-collapse, divide by row-sum, validity ----
            # csn collapses to numerator+denominator rows 0..2; csd
            # replicates the denominator onto rows 0..2 directly.
            prod = small.tile([M4P, NI], F16)
            s_res = small.tile([3, NI], F32)
            s_rg = small.tile([3, NI], F32)
            s_rln = small.tile([3, NI], F32)
            s_rcp = small.tile([3, NI], F32)
            for h in range(2):
                sl = slice(h * IW, (h + 1) * IW)
                nc.any.tensor_mul(prod[:, sl], ps_O4K[:, h, 0:IW],
                                  s_gl44[:, sl])
                ps_nd = psS.tile([3, 512], F32, tag="t", name="ps_nd")
                nc.tensor.matmul(ps_nd[:, 0:IW], s_csn[:], prod[:, sl],
                                 start=True, stop=True)
                ps_dn = psS.tile([3, 512], F32, tag="t", name="ps_dn")
                nc.tensor.matmul(ps_dn[:, 0:IW], s_csd[:], prod[:, sl],
                                 start=True, stop=True)
                # rcp = 1/max(den, 0.1): valid rows have den >= 2*e^-2,
                # invalid ones are zeroed by rv below
                nc.any.tensor_scalar(s_rg[:, sl], ps_dn[:, 0:IW], 0.1, None,
                                     ALU.max)
                nc.scalar.activation(s_rln[:, sl], s_rg[:, sl], AF.Ln,
                                     bias=b_zero[0:3, :])
                nc.scalar.activation(s_rcp[:, sl], s_rln[:, sl], AF.Exp,
                                     bias=b_zero[0:3, :], scale=-1.0)
                # (num+den)/den = out+1; multiply by validity, subtract 1
                nc.any.tensor_mul(s_res[:, sl], ps_nd[:, 0:IW],
                                  s_rcp[:, sl])
                nc.any.tensor_mul(s_res[:, sl], s_res[:, sl], s_rvf[:, sl])
                nc.any.tensor_scalar(s_res[:, sl], s_res[:, sl], -1.0, None,
                                     ALU.add)
            nc.sync.dma_start(d_out[:], s_res[:])

    return nc


# revision 22
# speedup vs baseline: 2.0580x; 1.0083x over previous
"""Trainium2 Bass kernel for nn_C_Net_77807627534400 (sparse_attention).

Reference semantics: for each batch image and each class k in 1..11, the
per-class masked-normalized gray/rgb features form an [N,N] correlation,
softmax over the rgb-mask pixels, and a weighted mean of the rgb image is
written at the gray-mask pixels (if both masks have >= 2 pixels).

Because every pixel belongs to exactly one class, the 11 per-class [N,N]
matmuls fuse into ONE [N,N] matmul of per-class-centered features. The
class-match mask is enforced EXACTLY in the output matmul: expand img4
(rgb + ones row) to 44 rows IMG4R[(c,k), j] = img4[c,j] * rl[k,j], so

    O4K[(c,k), i] = sum_j img4[c,j] rl[k,j] e[j,i]
    O4[c, i]      = sum_k gl[k,i] O4K[(c,k), i]     (per-i class select)

with e[j,i] = exp(corr[j,i] - 1) computed WITHOUT any masking bias. The
collapse is an elementwise multiply by gl44 (gl broadcast to 44 rows via a
tiny matmul) plus two [44 -> 3] summing matmuls per half: one produces
numerator+denominator, the other the denominator replicated onto partitions
0..2 (avoids any cross-partition moves in the tail).

Normalization: gray side is explicitly normalized and scaled by 16 into
fp8 range (unit16 = 16 * bar / ||bar||); the rgb side is NOT normalized --
raw centered bar_r is the matmul operand and rsqrt(ssq_r)/16 is applied as
the per-partition *scale* of the Exp activation. Per-j sumsq is computed in
j-partition layout with tiny N=1 matmuls. All rsqrt/reciprocal come from
exp(a*ln(x) + b) so ScalarE only ever loads the natural_log_exp_and_others
table set (exp/ln/copy/square live there) -- exactly one ACT_TABLE_LOAD.

Dtypes: fp8e4 (e4m3) for every large matmul operand; the big attention and
class-means matmuls run DoubleRow (K packed 2x128, 0.5 cycles/row). PSUM is
fp32; the softmax weighted-average structure keeps fp8 quantization noise
(~6% per element, averaged over ~450 mask pixels) far below the 2e-2
tolerance. All DRAM tensors are host-side laid out to exactly match their
SBUF tiles so every DMA is contiguous.

Sharding: 8 cores = 2 batches x 4 slices of 576 gray pixels. Each core
computes the full rgb side for its batch (redundant across 4 cores) and its
576-column slice of the gray side.
"""

import numpy as np

import concourse.bass as bass
import concourse.tile as tile
from concourse import mybir
from concourse.bass_utils import run_bass_kernel_spmd
from concourse.vector_clock import ScopedClock

B, C, H, W, NCH = 2, 256, 48, 48, 12
N = H * W           # 2304
NK = NCH - 1        # classes 1..11
QS = 4              # gray-pixel slices per batch
NI = N // QS        # 576 rows per core
NCORES = B * QS     # 8
JC = N // 128       # 18 j-chunks
JP = JC // 2        # 9 j-chunk pairs
CC = C // 128       # 2 c-chunks
IW = 288            # i-chunk width (two per slice)
RW = 256            # rgb normalize chunk width
NRC = N // RW       # 9 rgb chunks
M4 = 4 * NK         # 44 expanded img rows
M4P = 48            # M4 padded so DoubleRow plane strides are 16B-aligned
NKP = 16            # NK padded likewise for the transposed labels
F272 = 272          # 258 feature+count cols padded likewise
LN16 = float(np.log(16.0))
F32 = mybir.dt.float32
F16 = mybir.dt.float16
F8 = mybir.dt.float8e4
ALU = mybir.AluOpType
AF = mybir.ActivationFunctionType
DR = mybir.MatmulPerfMode.DoubleRow


class _TC(tile.TileContext):
    """Workaround: this walrus build rejects instructions carrying more than
    one sync-wait command. Split every multi-wait instruction into a chain of
    single-wait NOPs (same engine, program order preserved) followed by the
    original instruction holding the final wait."""

    def _add_instruction(self, inst):
        si = inst.sync_info
        if si is not None:
            waits = list(si.on_wait)
            if len(waits) > 1:
                nc = self.nc
                for w in waits[:-1]:
                    nop = mybir.InstNoOp(
                        name=nc.get_next_instruction_name(),
                        sync_info=mybir.SyncInfo(on_wait=[w], on_update=[]),
                        bass_nofuse=True,
                        engine=inst.engine,
                    )
                    super()._add_instruction(nop)
                si.on_wait = waits[-1:]
                inst.sync_info = si
        super()._add_instruction(inst)

    def _drain_and_barrier(self, tick_clock, wait_clock):
        nc = self.nc
        drain_inst = nc.sync.drain()
        wait_clock.add_sem_waits(
            drain_inst.ins, ScopedClock({None: tick_clock.global_clock})
        )
        si = drain_inst.ins.sync_info
        waits = list(si.on_wait) if si is not None else []
        if len(waits) > 1:
            si.on_wait = waits[:1]
            drain_inst.ins.sync_info = si
            for w in waits[1:]:
                extra = nc.sync.drain()
                extra.ins.sync_info = mybir.SyncInfo(on_wait=[w], on_update=[])

        nc.all_engine_barrier()
        assert self.sems is not None
        popped = nc._tile_sem_poison_stack.pop()
        assert popped is self._sem_poison
        nc.clear_and_free_semaphores(list(self.sems.allocated().values()))
        nc.all_engine_barrier()


def _build_nc():
    nc = bass.Bass(target_bir_lowering=False)

    d_glT = nc.dram_tensor("glT8", [128, JP, 2, NKP], F8, kind="ExternalInput")
    d_gfT = nc.dram_tensor("gfT8", [128, JP, 2, F272], F8, kind="ExternalInput")
    d_rlT = nc.dram_tensor("rlT8", [128, JP, 2, NKP], F8, kind="ExternalInput")
    d_rfT = nc.dram_tensor("rfT8", [128, JP, 2, F272], F8, kind="ExternalInput")
    d_gls = nc.dram_tensor("gls", [NK, NI], F8, kind="ExternalInput")
    d_gfs = nc.dram_tensor("gfs", [128, CC, NI], F8, kind="ExternalInput")
    d_rl = nc.dram_tensor("rl", [NK, N], F8, kind="ExternalInput")
    d_rf = nc.dram_tensor("rf", [128, CC, N], F8, kind="ExternalInput")
    d_i4r = nc.dram_tensor("i4r", [128, JP, 2, M4P], F8, kind="ExternalInput")
    d_kc = nc.dram_tensor("kc", [NK, M4P], F8, kind="ExternalInput")
    d_csn = nc.dram_tensor("csn", [M4P, 3], F16, kind="ExternalInput")
    d_csd = nc.dram_tensor("csd", [M4P, 3], F16, kind="ExternalInput")
    d_out = nc.dram_tensor("out", [3, NI], F32, kind="ExternalOutput")

    with _TC(nc) as tc:
        with (
            tc.tile_pool(name="big", bufs=1) as big,
            tc.tile_pool(name="work", bufs=1) as work,
            tc.tile_pool(name="chk", bufs=2) as chk,
            tc.tile_pool(name="expp", bufs=3) as expp,
            tc.tile_pool(name="small", bufs=1) as small,
            tc.tile_pool(name="psS", bufs=2, space="PSUM") as psS,
            tc.tile_pool(name="psM", bufs=2, space="PSUM") as psM,
            tc.tile_pool(name="psO", bufs=1, space="PSUM") as psO,
        ):
            # ---- loads, in consumption order ----
            s_glT = big.tile([128, JP, 2, NKP], F8)
            nc.sync.dma_start(s_glT[:], d_glT[:])
            s_gfT = big.tile([128, JP, 2, F272], F8)
            for p in range(0, JP, 3):
                nc.sync.dma_start(s_gfT[:, p:p + 3, :, :], d_gfT[:, p:p + 3, :, :])
            s_rlT = big.tile([128, JP, 2, NKP], F8)
            nc.sync.dma_start(s_rlT[:], d_rlT[:])
            s_rfT = big.tile([128, JP, 2, F272], F8)
            for p in range(0, JP, 3):
                nc.sync.dma_start(s_rfT[:, p:p + 3, :, :], d_rfT[:, p:p + 3, :, :])
            s_gls = big.tile([NK, NI], F8)
            nc.sync.dma_start(s_gls[:], d_gls[:])
            s_kc = big.tile([NK, M4P], F8)
            nc.sync.dma_start(s_kc[:], d_kc[:])
            s_gfs = big.tile([128, CC, NI], F8)
            nc.sync.dma_start(s_gfs[:], d_gfs[:])
            s_rl = big.tile([NK, N], F8)
            nc.sync.dma_start(s_rl[:], d_rl[:])
            s_rf = big.tile([128, CC, N], F8)
            for p in range(0, N, 1152):
                nc.sync.dma_start(s_rf[:, :, p:p + 1152],
                                  d_rf[:, :, p:p + 1152])
            s_i4r = big.tile([128, JP, 2, M4P], F8)
            nc.sync.dma_start(s_i4r[:], d_i4r[:])
            s_csn = big.tile([M4P, 3], F16)
            nc.sync.dma_start(s_csn[:], d_csn[:])
            s_csd = big.tile([M4P, 3], F16)
            nc.sync.dma_start(s_csd[:], d_csd[:])

            # on-chip constants
            s_ones16 = big.tile([128, 128], F16)
            nc.vector.memset(s_ones16[:], 1.0)
            b_zero = big.tile([128, 1], F32)
            nc.vector.memset(b_zero[:], 0.0)
            b_eps = big.tile([128, 1], F32)
            nc.vector.memset(b_eps[:], 1e-4)
            b_neg1 = big.tile([128, 1], F32)
            nc.vector.memset(b_neg1[:], -1.0)
            b_pln16 = big.tile([128, 1], F32)
            nc.vector.memset(b_pln16[:], LN16)
            b_nln16 = big.tile([128, 1], F32)
            nc.vector.memset(b_nln16[:], -LN16)

            # ---- per-class sums + counts (col 256 is the ones column) ----
            def class_means(s_lT, s_fT, nmtag):
                ps = psS.tile([NKP, 512], F32, tag="t", name=f"ps_mean{nmtag}")
                for p in range(JP):
                    nc.tensor.matmul(ps[:, 0:F272], s_lT[:, p, :, :],
                                     s_fT[:, p, :, :], perf_mode=DR,
                                     start=(p == 0), stop=(p == JP - 1))
                cnt = small.tile([NK, 1], F32, name=f"cnt{nmtag}")
                nc.vector.tensor_copy(cnt[:], ps[0:NK, 256:257])
                rc = small.tile([NK, 1], F32, name=f"rc{nmtag}")
                nc.vector.tensor_scalar(rc[:], cnt[:], 1.0, None, ALU.max)
                nc.vector.reciprocal(rc[:], rc[:])
                meanT = work.tile([NK, C], F8, name=f"mean{nmtag}")
                nc.vector.tensor_scalar(meanT[:], ps[0:NK, 0:C], rc[:], None,
                                        ALU.mult)
                return meanT, cnt

            meanT_g, cnt_g = class_means(s_glT, s_gfT, "g")
            meanT_r, cnt_r = class_means(s_rlT, s_rfT, "r")
            vg = small.tile([NK, 1], F32)
            nc.vector.tensor_scalar(vg[:], cnt_g[:], 1.5, None, ALU.is_gt)
            valid = small.tile([NK, 1], F32)
            nc.vector.tensor_scalar(valid[:], cnt_r[:], 1.5, None, ALU.is_gt)
            nc.vector.tensor_mul(valid[:], valid[:], vg[:])

            # gl44v[(c,k), i] = gl[k, i] * valid[k]: the per-class validity
            # rides the collapse multiply, so invalid rows make BOTH the
            # numerator and denominator zero -> out = 0/0.1*0 - 1 = -1.
            def emit_gl44v():
                kcv = small.tile([NK, M4P], F8, name="kcv")
                nc.vector.tensor_scalar(kcv[:], s_kc[:], valid[:], None,
                                        ALU.mult)
                for h in range(2):
                    sl = slice(h * IW, (h + 1) * IW)
                    ps = psS.tile([M4P, 512], F32, tag="t", name="ps_gl44")
                    nc.tensor.matmul(ps[:, 0:IW], kcv[:], s_gls[:, sl],
                                     start=True, stop=True)
                    nc.vector.tensor_copy(s_gl44[:, sl], ps[:, 0:IW])
            s_gl44 = small.tile([M4P, NI], F16)

            # ---- gray side: unit16_g = 16 * (gf - mu) / ||gf - mu|| ----
            unit_g = work.tile([128, CC, NI], F8, name="unitg")
            for ib in range(2):
                sl = slice(ib * IW, (ib + 1) * IW)
                barg = [chk.tile([128, IW], F16, tag=f"barg{cc}", bufs=2,
                                 name=f"barg{cc}") for cc in range(CC)]
                sqg = [chk.tile([128, IW], F16, tag=f"sqg{cc}", bufs=2,
                                name=f"sqg{cc}") for cc in range(CC)]
                for cc in range(CC):
                    ps = psS.tile([128, 512], F32, tag="t", name="ps_mug")
                    nc.tensor.matmul(ps[:, 0:IW],
                                     meanT_g[:, cc * 128:(cc + 1) * 128],
                                     s_gls[:, sl], start=True, stop=True)
                    nc.any.tensor_sub(barg[cc][:], s_gfs[:, cc, sl],
                                      ps[:, 0:IW])
                    nc.any.tensor_mul(sqg[cc][:], barg[cc][:], barg[cc][:])
                ps2 = psS.tile([128, 512], F32, tag="t", name="ps_ssqg")
                for cc in range(CC):
                    nc.tensor.matmul(ps2[:, 0:IW], s_ones16[:], sqg[cc][:],
                                     start=(cc == 0), stop=(cc == CC - 1))
                lng = chk.tile([128, IW], F32, tag="lng", bufs=2, name="lng")
                nc.scalar.activation(lng[:], ps2[:, 0:IW], AF.Ln,
                                     bias=b_eps[:])
                rbg = chk.tile([128, IW], F32, tag="rbg", bufs=2, name="rbg")
                nc.scalar.activation(rbg[:], lng[:], AF.Exp,
                                     bias=b_pln16[:], scale=-0.5)
                for cc in range(CC):
                    nc.any.tensor_mul(unit_g[:, cc, sl], barg[cc][:], rbg[:])

            # ---- rgb side: bar_r chunks (fp8, DoubleRow layout) + per-j
            # sumsq in j-partition layout; rsqrt/16 becomes the Exp scale ----
            bar_r = {}
            # rsqrt batches: A = chunks 0-1 (jc 0-3), B = chunks 2-4
            # (jc 4-9), C = chunks 5-8 (jc 10-17)
            ssq = [small.tile([128, 4], F32, name="ssqA"),
                   small.tile([128, 6], F32, name="ssqB"),
                   small.tile([128, 8], F32, name="ssqC")]
            rsq = [small.tile([128, 4], F32, name="rsqA"),
                   small.tile([128, 6], F32, name="rsqB"),
                   small.tile([128, 8], F32, name="rsqC")]
            BASE = [0, 4, 10]

            def batch_of(jc):
                bi = 0 if jc < 4 else (1 if jc < 10 else 2)
                return bi, jc - BASE[bi]

            def r_chunk(ib):
                sl = slice(ib * RW, (ib + 1) * RW)
                bar8 = chk.tile([128, 2, RW], F8, tag="bar8", bufs=10,
                                name="bar8")
                sq8 = chk.tile([128, 2, RW], F16, tag="sq8", bufs=3,
                               name="sq8")
                ps = psS.tile([128, 2, RW], F32, tag="t", name="ps_mur")
                for cc in range(CC):
                    nc.tensor.matmul(ps[:, cc, :],
                                     meanT_r[:, cc * 128:(cc + 1) * 128],
                                     s_rl[:, sl], start=True, stop=True)
                nc.any.tensor_sub(bar8[:], s_rf[:, :, sl], ps[:, :, :])
                nc.any.tensor_mul(sq8[:], bar8[:], bar8[:])
                for h in range(2):
                    jc = 2 * ib + h
                    lo = h * 128
                    ps2 = psS.tile([128, 512], F32, tag="t", name="ps_ssqr")
                    for cc in range(CC):
                        nc.tensor.matmul(ps2[:, 0:1], sq8[:, cc, lo:lo + 128],
                                         s_ones16[:, 0:1],
                                         start=(cc == 0), stop=(cc == CC - 1))
                    bi, col = batch_of(jc)
                    nc.vector.tensor_copy(ssq[bi][:, col:col + 1], ps2[:, 0:1])
                bar_r[ib] = bar8

            def rsqrt_batch(bi):
                w = ssq[bi].shape[1]
                t = small.tile([128, 8], F32, name=f"lnr{bi}")
                nc.scalar.activation(t[:, 0:w], ssq[bi][:], AF.Ln,
                                     bias=b_eps[:])
                nc.scalar.activation(rsq[bi][:], t[:, 0:w], AF.Exp,
                                     bias=b_nln16[:], scale=-0.5)

            # ---- attention pairs + masked-output accumulation ----
            ps_O4K = psO.tile([M4P, 2, 512], F32)

            def attention_pair(pr):
                s_exp = expp.tile([128, 2, NI], F8, tag="exp", name="s_exp")
                for h in range(2):
                    jc = 2 * pr + h
                    ib, lo = jc // 2, (jc % 2) * 128
                    bar8 = bar_r[ib]
                    ps_mt = psM.tile([128, 2, 512], F32, tag="mt",
                                     name="ps_mt")
                    for ic in range(2):
                        i0 = ic * IW
                        nc.tensor.matmul(ps_mt[:, ic, 0:IW],
                                         bar8[:, :, lo:lo + 128],
                                         unit_g[:, :, i0:i0 + IW],
                                         perf_mode=DR, start=True, stop=True)
                    bi, col = batch_of(jc)
                    nc.scalar.activation(
                        s_exp[:, h, :].rearrange("p (a b) -> p a b", a=2),
                        ps_mt[:, :, 0:IW], AF.Exp, bias=b_neg1[:],
                        scale=rsq[bi][:, col:col + 1])
                for ic in range(2):
                    i0 = ic * IW
                    nc.tensor.matmul(ps_O4K[:, ic, 0:IW], s_i4r[:, pr, :, :],
                                     s_exp[:, :, i0:i0 + IW], perf_mode=DR,
                                     start=(pr == 0), stop=(pr == JP - 1))

            # schedule: chunks 0-1 up front unlock pairs 0-1 (jc 0-3);
            # chunks 2-4 + batch B are emitted during pairs 0-1 (B is read
            # from pair 2 = jc 4); chunks 5-8 + batch C during pairs 2-4
            # (C is read from pair 5 = jc 10)
            r_chunk(0)
            r_chunk(1)
            rsqrt_batch(0)
            NEXT = {0: [2, 3], 1: [4], 2: [5], 3: [6], 4: [7, 8]}
            for pr in range(JP):
                attention_pair(pr)
                for nxt in NEXT.get(pr, []):
                    r_chunk(nxt)
                    if nxt == 4:
                        rsqrt_batch(1)
                    if nxt == 8:
                        rsqrt_batch(2)
                if pr == 0:
                    emit_gl44v()

            # ---- finalize: class-collapse, divide by row-sum, validity ----
            # csn collapses to numerator+denominator rows 0..2; csd
            # replicates the denominator onto rows 0..2 directly.
            prod = small.tile([M4P, NI], F16)
            s_res = small.tile([3, NI], F32)
            s_rg = small.tile([3, NI], F32)
            s_rln = small.tile([3, NI], F32)
            s_rcp = small.tile([3, NI], F32)
            for h in range(2):
                sl = slice(h * IW, (h + 1) * IW)
                nc.any.tensor_mul(prod[:, sl], ps_O4K[:, h, 0:IW],
                                  s_gl44[:, sl])
                ps_nd = psS.tile([3, 512], F32, tag="t", name="ps_nd")
                nc.tensor.matmul(ps_nd[:, 0:IW], s_csn[:], prod[:, sl],
                                 start=True, stop=True)
                ps_dn = psS.tile([3, 512], F32, tag="t", name="ps_dn")
                nc.tensor.matmul(ps_dn[:, 0:IW], s_csd[:], prod[:, sl],
                                 start=True, stop=True)
                # rcp = 1/max(den, 0.1): valid rows have den >= 2*e^-2,
                # invalid ones are zeroed by rv below
                nc.any.tensor_scalar(s_rg[:, sl], ps_dn[:, 0:IW], 0.1, None,
                                     ALU.max)
                nc.scalar.activation(s_rln[:, sl], s_rg[:, sl], AF.Ln,
                                     bias=b_zero[0:3, :])
                nc.scalar.activation(s_rcp[:, sl], s_rln[:, sl], AF.Exp,
                                     bias=b_zero[0:3, :], scale=-1.0)
                # (num+den)/den = out+1; multiply by validity, subtract 1
                nc.vector.scalar_tensor_tensor(
                    s_res[:, sl], ps_nd[:, 0:IW], 1.0, s_rcp[:, sl],
                    ALU.mult, ALU.mult)
                nc.any.tensor_scalar(s_res[:, sl], s_res[:, sl], -1.0, None,
                                     ALU.add)
            nc.sync.dma_start(d_out[:], s_res[:])

    return nc


# revision 23
# speedup vs baseline: 2.1112x; 1.0259x over previous
"""Trainium2 Bass kernel for nn_C_Net_77807627534400 (sparse_attention).

Reference semantics: for each batch image and each class k in 1..11, the
per-class masked-normalized gray/rgb features form an [N,N] correlation,
softmax over the rgb-mask pixels, and a weighted mean of the rgb image is
written at the gray-mask pixels (if both masks have >= 2 pixels).

Because every pixel belongs to exactly one class, the 11 per-class [N,N]
matmuls fuse into ONE [N,N] matmul of per-class-centered features. The
class-match mask is enforced EXACTLY in the output matmul: expand img4
(rgb + ones row) to 44 rows IMG4R[(c,k), j] = img4[c,j] * rl[k,j], so

    O4K[(c,k), i] = sum_j img4[c,j] rl[k,j] e[j,i]
    O4[c, i]      = sum_k gl[k,i] O4K[(c,k), i]     (per-i class select)

with e[j,i] = exp(corr[j,i] - 1) computed WITHOUT any masking bias. The
collapse is an elementwise multiply by gl44 (gl broadcast to 44 rows via a
tiny matmul) plus two [44 -> 3] summing matmuls per half: one produces
numerator+denominator, the other the denominator replicated onto partitions
0..2 (avoids any cross-partition moves in the tail).

Normalization: gray side is explicitly normalized and scaled by 16 into
fp8 range (unit16 = 16 * bar / ||bar||); the rgb side is NOT normalized --
raw centered bar_r is the matmul operand and rsqrt(ssq_r)/16 is applied as
the per-partition *scale* of the Exp activation. Per-j sumsq is computed in
j-partition layout with tiny N=1 matmuls. All rsqrt/reciprocal come from
exp(a*ln(x) + b) so ScalarE only ever loads the natural_log_exp_and_others
table set (exp/ln/copy/square live there) -- exactly one ACT_TABLE_LOAD.

Dtypes: fp8e4 (e4m3) for every large matmul operand; the big attention and
class-means matmuls run DoubleRow (K packed 2x128, 0.5 cycles/row). PSUM is
fp32; the softmax weighted-average structure keeps fp8 quantization noise
(~6% per element, averaged over ~450 mask pixels) far below the 2e-2
tolerance. All DRAM tensors are host-side laid out to exactly match their
SBUF tiles so every DMA is contiguous.

Sharding: 8 cores = 2 batches x 4 slices of 576 gray pixels. Each core
computes the full rgb side for its batch (redundant across 4 cores) and its
576-column slice of the gray side.
"""

import numpy as np

import concourse.bass as bass
import concourse.tile as tile
from concourse import mybir
from concourse.bass_utils import run_bass_kernel_spmd
from concourse.vector_clock import ScopedClock

B, C, H, W, NCH = 2, 256, 48, 48, 12
N = H * W           # 2304
NK = NCH - 1        # classes 1..11
QS = 4              # gray-pixel slices per batch
NI = N // QS        # 576 rows per core
NCORES = B * QS     # 8
JC = N // 128       # 18 j-chunks
JP = JC // 2        # 9 j-chunk pairs
CC = C // 128       # 2 c-chunks
IW = 288            # i-chunk width (two per slice)
RW = 256            # rgb normalize chunk width
NRC = N // RW       # 9 rgb chunks
M4 = 4 * NK         # 44 expanded img rows
M4P = 48            # M4 padded so DoubleRow plane strides are 16B-aligned
NKP = 16            # NK padded likewise for the transposed labels
F272 = 272          # 258 feature+count cols padded likewise
LN16 = float(np.log(16.0))
F32 = mybir.dt.float32
F16 = mybir.dt.float16
F8 = mybir.dt.float8e4
ALU = mybir.AluOpType
AF = mybir.ActivationFunctionType
DR = mybir.MatmulPerfMode.DoubleRow


class _TC(tile.TileContext):
    """Workaround: this walrus build rejects instructions carrying more than
    one sync-wait command. Split every multi-wait instruction into a chain of
    single-wait NOPs (same engine, program order preserved) followed by the
    original instruction holding the final wait."""

    def _add_instruction(self, inst):
        si = inst.sync_info
        if si is not None:
            waits = list(si.on_wait)
            if len(waits) > 1:
                nc = self.nc
                for w in waits[:-1]:
                    nop = mybir.InstNoOp(
                        name=nc.get_next_instruction_name(),
                        sync_info=mybir.SyncInfo(on_wait=[w], on_update=[]),
                        bass_nofuse=True,
                        engine=inst.engine,
                    )
                    super()._add_instruction(nop)
                si.on_wait = waits[-1:]
                inst.sync_info = si
        super()._add_instruction(inst)

    def _drain_and_barrier(self, tick_clock, wait_clock):
        nc = self.nc
        drain_inst = nc.sync.drain()
        wait_clock.add_sem_waits(
            drain_inst.ins, ScopedClock({None: tick_clock.global_clock})
        )
        si = drain_inst.ins.sync_info
        waits = list(si.on_wait) if si is not None else []
        if len(waits) > 1:
            si.on_wait = waits[:1]
            drain_inst.ins.sync_info = si
            for w in waits[1:]:
                extra = nc.sync.drain()
                extra.ins.sync_info = mybir.SyncInfo(on_wait=[w], on_update=[])

        nc.all_engine_barrier()
        assert self.sems is not None
        popped = nc._tile_sem_poison_stack.pop()
        assert popped is self._sem_poison
        nc.clear_and_free_semaphores(list(self.sems.allocated().values()))
        nc.all_engine_barrier()


def _build_nc():
    nc = bass.Bass(target_bir_lowering=False)

    d_glT = nc.dram_tensor("glT8", [128, JP, 2, NKP], F8, kind="ExternalInput")
    d_gfT = nc.dram_tensor("gfT8", [128, JP, 2, F272], F8, kind="ExternalInput")
    d_rlT = nc.dram_tensor("rlT8", [128, JP, 2, NKP], F8, kind="ExternalInput")
    d_rfT = nc.dram_tensor("rfT8", [128, JP, 2, F272], F8, kind="ExternalInput")
    d_gls = nc.dram_tensor("gls", [NK, NI], F8, kind="ExternalInput")
    d_gfs = nc.dram_tensor("gfs", [128, CC, NI], F8, kind="ExternalInput")
    d_rl = nc.dram_tensor("rl", [NK, N], F8, kind="ExternalInput")
    d_rf = nc.dram_tensor("rf", [128, CC, N], F8, kind="ExternalInput")
    d_i4r = nc.dram_tensor("i4r", [128, JP, 2, M4P], F8, kind="ExternalInput")
    d_kc = nc.dram_tensor("kc", [NK, M4P], F8, kind="ExternalInput")
    d_csn = nc.dram_tensor("csn", [M4P, 3], F16, kind="ExternalInput")
    d_csd = nc.dram_tensor("csd", [M4P, 3], F16, kind="ExternalInput")
    d_out = nc.dram_tensor("out", [3, NI], F32, kind="ExternalOutput")

    with _TC(nc) as tc:
        with (
            tc.tile_pool(name="big", bufs=1) as big,
            tc.tile_pool(name="work", bufs=1) as work,
            tc.tile_pool(name="chk", bufs=2) as chk,
            tc.tile_pool(name="expp", bufs=3) as expp,
            tc.tile_pool(name="small", bufs=1) as small,
            tc.tile_pool(name="psS", bufs=2, space="PSUM") as psS,
            tc.tile_pool(name="psM", bufs=2, space="PSUM") as psM,
            tc.tile_pool(name="psO", bufs=1, space="PSUM") as psO,
        ):
            # ---- loads, in consumption order ----
            s_glT = big.tile([128, JP, 2, NKP], F8)
            nc.sync.dma_start(s_glT[:], d_glT[:])
            s_gfT = big.tile([128, JP, 2, F272], F8)
            for p in range(0, JP, 3):
                nc.sync.dma_start(s_gfT[:, p:p + 3, :, :], d_gfT[:, p:p + 3, :, :])
            s_rlT = big.tile([128, JP, 2, NKP], F8)
            nc.sync.dma_start(s_rlT[:], d_rlT[:])
            s_rfT = big.tile([128, JP, 2, F272], F8)
            for p in range(0, JP, 3):
                nc.sync.dma_start(s_rfT[:, p:p + 3, :, :], d_rfT[:, p:p + 3, :, :])
            s_gls = big.tile([NK, NI], F8)
            nc.sync.dma_start(s_gls[:], d_gls[:])
            s_kc = big.tile([NK, M4P], F8)
            nc.sync.dma_start(s_kc[:], d_kc[:])
            s_gfs = big.tile([128, CC, NI], F8)
            nc.sync.dma_start(s_gfs[:], d_gfs[:])
            s_rl = big.tile([NK, N], F8)
            nc.sync.dma_start(s_rl[:], d_rl[:])
            s_rf = big.tile([128, CC, N], F8)
            for p in range(0, N, 1152):
                nc.sync.dma_start(s_rf[:, :, p:p + 1152],
                                  d_rf[:, :, p:p + 1152])
            s_i4r = big.tile([128, JP, 2, M4P], F8)
            nc.sync.dma_start(s_i4r[:], d_i4r[:])
            s_csn = big.tile([M4P, 3], F16)
            nc.sync.dma_start(s_csn[:], d_csn[:])
            s_csd = big.tile([M4P, 3], F16)
            nc.sync.dma_start(s_csd[:], d_csd[:])

            # on-chip constants
            s_ones16 = big.tile([128, 128], F16)
            nc.vector.memset(s_ones16[:], 1.0)
            b_zero = big.tile([128, 1], F32)
            nc.vector.memset(b_zero[:], 0.0)
            b_eps = big.tile([128, 1], F32)
            nc.vector.memset(b_eps[:], 1e-4)
            b_neg1 = big.tile([128, 1], F32)
            nc.vector.memset(b_neg1[:], -1.0)
            b_pln16 = big.tile([128, 1], F32)
            nc.vector.memset(b_pln16[:], LN16)
            b_nln16 = big.tile([128, 1], F32)
            nc.vector.memset(b_nln16[:], -LN16)

            # ---- per-class sums + counts (col 256 is the ones column) ----
            def class_means(s_lT, s_fT, nmtag):
                ps = psS.tile([NKP, 512], F32, tag="t", name=f"ps_mean{nmtag}")
                for p in range(JP):
                    nc.tensor.matmul(ps[:, 0:F272], s_lT[:, p, :, :],
                                     s_fT[:, p, :, :], perf_mode=DR,
                                     start=(p == 0), stop=(p == JP - 1))
                cnt = small.tile([NK, 1], F32, name=f"cnt{nmtag}")
                nc.vector.tensor_copy(cnt[:], ps[0:NK, 256:257])
                rc = small.tile([NK, 1], F32, name=f"rc{nmtag}")
                nc.vector.tensor_scalar(rc[:], cnt[:], 1.0, None, ALU.max)
                nc.vector.reciprocal(rc[:], rc[:])
                meanT = work.tile([NK, C], F8, name=f"mean{nmtag}")
                nc.vector.tensor_scalar(meanT[:], ps[0:NK, 0:C], rc[:], None,
                                        ALU.mult)
                return meanT, cnt

            meanT_g, cnt_g = class_means(s_glT, s_gfT, "g")
            meanT_r, cnt_r = class_means(s_rlT, s_rfT, "r")
            vg = small.tile([NK, 1], F32)
            nc.vector.tensor_scalar(vg[:], cnt_g[:], 1.5, None, ALU.is_gt)
            valid = small.tile([NK, 1], F32)
            nc.vector.tensor_scalar(valid[:], cnt_r[:], 1.5, None, ALU.is_gt)
            nc.vector.tensor_mul(valid[:], valid[:], vg[:])

            # gl44v[(c,k), i] = gl[k, i] * valid[k]: the per-class validity
            # rides the collapse multiply, so invalid rows make BOTH the
            # numerator and denominator zero -> out = 0/0.1*0 - 1 = -1.
            def emit_gl44v():
                kcv = small.tile([NK, M4P], F8, name="kcv")
                nc.vector.tensor_scalar(kcv[:], s_kc[:], valid[:], None,
                                        ALU.mult)
                for h in range(2):
                    sl = slice(h * IW, (h + 1) * IW)
                    ps = psS.tile([M4P, 512], F32, tag="t", name="ps_gl44")
                    nc.tensor.matmul(ps[:, 0:IW], kcv[:], s_gls[:, sl],
                                     start=True, stop=True)
                    nc.vector.tensor_copy(s_gl44[:, sl], ps[:, 0:IW])
            s_gl44 = small.tile([M4P, NI], F16)

            # ---- gray side: unit16_g = 16 * (gf - mu) / ||gf - mu|| ----
            unit_g = [work.tile([128, CC, IW], F8, name="unitg0"),
                      work.tile([128, CC, IW], F8, name="unitg1")]
            for ib in range(2):
                sl = slice(ib * IW, (ib + 1) * IW)
                barg = [chk.tile([128, IW], F16, tag=f"barg{cc}", bufs=2,
                                 name=f"barg{cc}") for cc in range(CC)]
                sqg = [chk.tile([128, IW], F16, tag=f"sqg{cc}", bufs=2,
                                name=f"sqg{cc}") for cc in range(CC)]
                for cc in range(CC):
                    ps = psS.tile([128, 512], F32, tag="t", name="ps_mug")
                    nc.tensor.matmul(ps[:, 0:IW],
                                     meanT_g[:, cc * 128:(cc + 1) * 128],
                                     s_gls[:, sl], start=True, stop=True)
                    nc.any.tensor_sub(barg[cc][:], s_gfs[:, cc, sl],
                                      ps[:, 0:IW])
                    nc.any.tensor_mul(sqg[cc][:], barg[cc][:], barg[cc][:])
                ps2 = psS.tile([128, 512], F32, tag="t", name="ps_ssqg")
                for cc in range(CC):
                    nc.tensor.matmul(ps2[:, 0:IW], s_ones16[:], sqg[cc][:],
                                     start=(cc == 0), stop=(cc == CC - 1))
                lng = chk.tile([128, IW], F32, tag="lng", bufs=2, name="lng")
                nc.scalar.activation(lng[:], ps2[:, 0:IW], AF.Ln,
                                     bias=b_eps[:])
                rbg = chk.tile([128, IW], F32, tag="rbg", bufs=2, name="rbg")
                nc.scalar.activation(rbg[:], lng[:], AF.Exp,
                                     bias=b_pln16[:], scale=-0.5)
                for cc in range(CC):
                    nc.any.tensor_mul(unit_g[ib][:, cc, :], barg[cc][:],
                                      rbg[:])

            # ---- rgb side: bar_r chunks (fp8, DoubleRow layout) + per-j
            # sumsq in j-partition layout; rsqrt/16 becomes the Exp scale ----
            bar_r = {}
            # rsqrt batches: A = chunks 0-1 (jc 0-3), B = chunks 2-4
            # (jc 4-9), C = chunks 5-8 (jc 10-17)
            ssq = [small.tile([128, 2], F32, name="ssqA0"),
                   small.tile([128, 2], F32, name="ssqA1"),
                   small.tile([128, 6], F32, name="ssqB"),
                   small.tile([128, 8], F32, name="ssqC")]
            rsq = [small.tile([128, 2], F32, name="rsqA0"),
                   small.tile([128, 2], F32, name="rsqA1"),
                   small.tile([128, 6], F32, name="rsqB"),
                   small.tile([128, 8], F32, name="rsqC")]
            BASE = [0, 2, 4, 10]

            def batch_of(jc):
                bi = 0 if jc < 2 else (1 if jc < 4 else (2 if jc < 10 else 3))
                return bi, jc - BASE[bi]

            def r_chunk(ib):
                sl = slice(ib * RW, (ib + 1) * RW)
                bar8 = chk.tile([128, 2, RW], F8, tag="bar8", bufs=10,
                                name="bar8")
                sq8 = chk.tile([128, 2, RW], F16, tag="sq8", bufs=3,
                               name="sq8")
                ps = psS.tile([128, 2, RW], F32, tag="t", name="ps_mur")
                for cc in range(CC):
                    nc.tensor.matmul(ps[:, cc, :],
                                     meanT_r[:, cc * 128:(cc + 1) * 128],
                                     s_rl[:, sl], start=True, stop=True)
                nc.any.tensor_sub(bar8[:], s_rf[:, :, sl], ps[:, :, :])
                nc.any.tensor_mul(sq8[:], bar8[:], bar8[:])
                for h in range(2):
                    jc = 2 * ib + h
                    lo = h * 128
                    ps2 = psS.tile([128, 512], F32, tag="t", name="ps_ssqr")
                    for cc in range(CC):
                        nc.tensor.matmul(ps2[:, 0:1], sq8[:, cc, lo:lo + 128],
                                         s_ones16[:, 0:1],
                                         start=(cc == 0), stop=(cc == CC - 1))
                    bi, col = batch_of(jc)
                    nc.vector.tensor_copy(ssq[bi][:, col:col + 1], ps2[:, 0:1])
                bar_r[ib] = bar8

            def rsqrt_batch(bi):
                w = ssq[bi].shape[1]
                t = small.tile([128, 8], F32, name=f"lnr{bi}")
                nc.scalar.activation(t[:, 0:w], ssq[bi][:], AF.Ln,
                                     bias=b_eps[:])
                nc.scalar.activation(rsq[bi][:], t[:, 0:w], AF.Exp,
                                     bias=b_nln16[:], scale=-0.5)

            # ---- attention pairs + masked-output accumulation ----
            ps_O4K = psO.tile([M4P, 2, 512], F32)

            def attention_pair(pr):
                s_exp = expp.tile([128, 2, NI], F8, tag="exp", name="s_exp")
                for h in range(2):
                    jc = 2 * pr + h
                    ib, lo = jc // 2, (jc % 2) * 128
                    bar8 = bar_r[ib]
                    ps_mt = psM.tile([128, 2, 512], F32, tag="mt",
                                     name="ps_mt")
                    for ic in range(2):
                        nc.tensor.matmul(ps_mt[:, ic, 0:IW],
                                         bar8[:, :, lo:lo + 128],
                                         unit_g[ic][:, :, :],
                                         perf_mode=DR, start=True, stop=True)
                    bi, col = batch_of(jc)
                    nc.scalar.activation(
                        s_exp[:, h, :].rearrange("p (a b) -> p a b", a=2),
                        ps_mt[:, :, 0:IW], AF.Exp, bias=b_neg1[:],
                        scale=rsq[bi][:, col:col + 1])
                for ic in range(2):
                    i0 = ic * IW
                    nc.tensor.matmul(ps_O4K[:, ic, 0:IW], s_i4r[:, pr, :, :],
                                     s_exp[:, :, i0:i0 + IW], perf_mode=DR,
                                     start=(pr == 0), stop=(pr == JP - 1))

            # schedule: chunks 0-1 up front unlock pairs 0-1 (jc 0-3);
            # chunks 2-4 + batch B are emitted during pairs 0-1 (B is read
            # from pair 2 = jc 4); chunks 5-8 + batch C during pairs 2-4
            # (C is read from pair 5 = jc 10)
            r_chunk(0)
            rsqrt_batch(0)
            r_chunk(1)
            rsqrt_batch(1)
            NEXT = {0: [2, 3], 1: [4], 2: [5], 3: [6], 4: [7, 8]}
            for pr in range(JP):
                attention_pair(pr)
                for nxt in NEXT.get(pr, []):
                    r_chunk(nxt)
                    if nxt == 4:
                        rsqrt_batch(2)
                    if nxt == 8:
                        rsqrt_batch(3)
                if pr == 0:
                    emit_gl44v()

            # ---- finalize: class-collapse, divide by row-sum, validity ----
            # csn collapses to numerator+denominator rows 0..2; csd
            # replicates the denominator onto rows 0..2 directly.
            prod = small.tile([M4P, NI], F16)
            s_res = small.tile([3, NI], F32)
            s_rg = small.tile([3, NI], F32)
            s_rln = small.tile([3, NI], F32)
            s_rcp = small.tile([3, NI], F32)
            for h in range(2):
                sl = slice(h * IW, (h + 1) * IW)
                nc.any.tensor_mul(prod[:, sl], ps_O4K[:, h, 0:IW],
                                  s_gl44[:, sl])
                ps_nd = psS.tile([3, 512], F32, tag="t", name="ps_nd")
                nc.tensor.matmul(ps_nd[:, 0:IW], s_csn[:], prod[:, sl],
                                 start=True, stop=True)
                ps_dn = psS.tile([3, 512], F32, tag="t", name="ps_dn")
                nc.tensor.matmul(ps_dn[:, 0:IW], s_csd[:], prod[:, sl],
                                 start=True, stop=True)
                # rcp = 1/max(den, 0.1): valid rows have den >= 2*e^-2,
                # invalid ones are zeroed by rv below
                nc.any.tensor_scalar(s_rg[:, sl], ps_dn[:, 0:IW], 0.1, None,
                                     ALU.max)
                nc.scalar.activation(s_rln[:, sl], s_rg[:, sl], AF.Ln,
                                     bias=b_zero[0:3, :])
                nc.scalar.activation(s_rcp[:, sl], s_rln[:, sl], AF.Exp,
                                     bias=b_zero[0:3, :], scale=-1.0)
                # (num+den)/den = out+1; multiply by validity, subtract 1
                nc.vector.scalar_tensor_tensor(
                    s_res[:, sl], ps_nd[:, 0:IW], 1.0, s_rcp[:, sl],
                    ALU.mult, ALU.mult)
                nc.any.tensor_scalar(s_res[:, sl], s_res[:, sl], -1.0, None,
                                     ALU.add)
            nc.sync.dma_start(d_out[:], s_res[:])

    return nc


# revision 24
# speedup vs baseline: 2.1854x; 1.0351x over previous
"""Trainium2 Bass kernel for nn_C_Net_77807627534400 (sparse_attention).

Reference semantics: for each batch image and each class k in 1..11, the
per-class masked-normalized gray/rgb features form an [N,N] correlation,
softmax over the rgb-mask pixels, and a weighted mean of the rgb image is
written at the gray-mask pixels (if both masks have >= 2 pixels).

Because every pixel belongs to exactly one class, the 11 per-class [N,N]
matmuls fuse into ONE [N,N] matmul of per-class-centered features. The
class-match mask is enforced EXACTLY in the output matmul: expand img4
(rgb + ones row) to 44 rows IMG4R[(c,k), j] = img4[c,j] * rl[k,j], so

    O4K[(c,k), i] = sum_j img4[c,j] rl[k,j] e[j,i]
    O4[c, i]      = sum_k gl[k,i] O4K[(c,k), i]     (per-i class select)

with e[j,i] = exp(corr[j,i] - 1) computed WITHOUT any masking bias. The
collapse is an elementwise multiply by gl44 (gl broadcast to 44 rows via a
tiny matmul) plus two [44 -> 3] summing matmuls per half: one produces
numerator+denominator, the other the denominator replicated onto partitions
0..2 (avoids any cross-partition moves in the tail).

Normalization: gray side is explicitly normalized and scaled by 16 into
fp8 range (unit16 = 16 * bar / ||bar||); the rgb side is NOT normalized --
raw centered bar_r is the matmul operand and rsqrt(ssq_r)/16 is applied as
the per-partition *scale* of the Exp activation. Per-j sumsq is computed in
j-partition layout with tiny N=1 matmuls. All rsqrt/reciprocal come from
exp(a*ln(x) + b) so ScalarE only ever loads the natural_log_exp_and_others
table set (exp/ln/copy/square live there) -- exactly one ACT_TABLE_LOAD.

Dtypes: fp8e4 (e4m3) for every large matmul operand; the big attention and
class-means matmuls run DoubleRow (K packed 2x128, 0.5 cycles/row). PSUM is
fp32; the softmax weighted-average structure keeps fp8 quantization noise
(~6% per element, averaged over ~450 mask pixels) far below the 2e-2
tolerance. All DRAM tensors are host-side laid out to exactly match their
SBUF tiles so every DMA is contiguous.

Sharding: 8 cores = 2 batches x 4 slices of 576 gray pixels. Each core
computes the full rgb side for its batch (redundant across 4 cores) and its
576-column slice of the gray side.
"""

import numpy as np

import concourse.bass as bass
import concourse.tile as tile
from concourse import mybir
from concourse.bass_utils import run_bass_kernel_spmd
from concourse.vector_clock import ScopedClock

B, C, H, W, NCH = 2, 256, 48, 48, 12
N = H * W           # 2304
NK = NCH - 1        # classes 1..11
QS = 4              # gray-pixel slices per batch
NI = N // QS        # 576 rows per core
NCORES = B * QS     # 8
JC = N // 128       # 18 j-chunks
JP = JC // 2        # 9 j-chunk pairs
CC = C // 128       # 2 c-chunks
IW = 288            # i-chunk width (two per slice)
RW = 256            # rgb normalize chunk width
NRC = N // RW       # 9 rgb chunks
M4 = 4 * NK         # 44 expanded img rows
M4P = 48            # M4 padded so DoubleRow plane strides are 16B-aligned
NKP = 16            # NK padded likewise for the transposed labels
F272 = 272          # 258 feature+count cols padded likewise
LN16 = float(np.log(16.0))
F32 = mybir.dt.float32
F16 = mybir.dt.float16
F8 = mybir.dt.float8e4
ALU = mybir.AluOpType
AF = mybir.ActivationFunctionType
DR = mybir.MatmulPerfMode.DoubleRow


class _TC(tile.TileContext):
    """Workaround: this walrus build rejects instructions carrying more than
    one sync-wait command. Split every multi-wait instruction into a chain of
    single-wait NOPs (same engine, program order preserved) followed by the
    original instruction holding the final wait."""

    def _add_instruction(self, inst):
        si = inst.sync_info
        if si is not None:
            waits = list(si.on_wait)
            if len(waits) > 1:
                nc = self.nc
                for w in waits[:-1]:
                    nop = mybir.InstNoOp(
                        name=nc.get_next_instruction_name(),
                        sync_info=mybir.SyncInfo(on_wait=[w], on_update=[]),
                        bass_nofuse=True,
                        engine=inst.engine,
                    )
                    super()._add_instruction(nop)
                si.on_wait = waits[-1:]
                inst.sync_info = si
        super()._add_instruction(inst)

    def _drain_and_barrier(self, tick_clock, wait_clock):
        nc = self.nc
        drain_inst = nc.sync.drain()
        wait_clock.add_sem_waits(
            drain_inst.ins, ScopedClock({None: tick_clock.global_clock})
        )
        si = drain_inst.ins.sync_info
        waits = list(si.on_wait) if si is not None else []
        if len(waits) > 1:
            si.on_wait = waits[:1]
            drain_inst.ins.sync_info = si
            for w in waits[1:]:
                extra = nc.sync.drain()
                extra.ins.sync_info = mybir.SyncInfo(on_wait=[w], on_update=[])

        nc.all_engine_barrier()
        assert self.sems is not None
        popped = nc._tile_sem_poison_stack.pop()
        assert popped is self._sem_poison
        nc.clear_and_free_semaphores(list(self.sems.allocated().values()))
        nc.all_engine_barrier()


def _build_nc():
    nc = bass.Bass(target_bir_lowering=False)

    d_glT = nc.dram_tensor("glT8", [128, JP, 2, NKP], F8, kind="ExternalInput")
    d_gfT = nc.dram_tensor("gfT8", [128, JP, 2, F272], F8, kind="ExternalInput")
    d_rlT = nc.dram_tensor("rlT8", [128, JP, 2, NKP], F8, kind="ExternalInput")
    d_rfT = nc.dram_tensor("rfT8", [128, JP, 2, F272], F8, kind="ExternalInput")
    d_gls = nc.dram_tensor("gls", [NK, NI], F8, kind="ExternalInput")
    d_gfs = nc.dram_tensor("gfs", [128, CC, NI], F8, kind="ExternalInput")
    d_rl = nc.dram_tensor("rl", [NK, N], F8, kind="ExternalInput")
    d_rf = nc.dram_tensor("rf", [128, CC, N], F8, kind="ExternalInput")
    d_i4r = nc.dram_tensor("i4r", [128, JP, 2, M4P], F8, kind="ExternalInput")
    d_kc = nc.dram_tensor("kc", [NK, M4P], F8, kind="ExternalInput")
    d_csn = nc.dram_tensor("csn", [M4P, 3], F16, kind="ExternalInput")
    d_csd = nc.dram_tensor("csd", [M4P, 3], F16, kind="ExternalInput")
    d_out = nc.dram_tensor("out", [3, NI], F32, kind="ExternalOutput")

    with _TC(nc) as tc:
        with (
            tc.tile_pool(name="big", bufs=1) as big,
            tc.tile_pool(name="work", bufs=1) as work,
            tc.tile_pool(name="chk", bufs=2) as chk,
            tc.tile_pool(name="expp", bufs=3) as expp,
            tc.tile_pool(name="small", bufs=1) as small,
            tc.tile_pool(name="psS", bufs=2, space="PSUM") as psS,
            tc.tile_pool(name="psM", bufs=2, space="PSUM") as psM,
            tc.tile_pool(name="psO", bufs=1, space="PSUM") as psO,
        ):
            # ---- loads, in consumption order ----
            s_glT = big.tile([128, JP, 2, NKP], F8)
            nc.sync.dma_start(s_glT[:], d_glT[:])
            s_rlT = big.tile([128, JP, 2, NKP], F8)
            nc.sync.dma_start(s_rlT[:], d_rlT[:])
            s_gfT = big.tile([128, JP, 2, F272], F8)
            s_rfT = big.tile([128, JP, 2, F272], F8)
            for p in range(0, JP, 3):
                nc.sync.dma_start(s_gfT[:, p:p + 3, :, :], d_gfT[:, p:p + 3, :, :])
                nc.sync.dma_start(s_rfT[:, p:p + 3, :, :], d_rfT[:, p:p + 3, :, :])
            s_gls = big.tile([NK, NI], F8)
            nc.sync.dma_start(s_gls[:], d_gls[:])
            s_kc = big.tile([NK, M4P], F8)
            nc.sync.dma_start(s_kc[:], d_kc[:])
            s_gfs = big.tile([128, CC, NI], F8)
            nc.sync.dma_start(s_gfs[:], d_gfs[:])
            s_rl = big.tile([NK, N], F8)
            nc.sync.dma_start(s_rl[:], d_rl[:])
            s_rf = big.tile([128, CC, N], F8)
            for p in range(0, N, 1152):
                nc.sync.dma_start(s_rf[:, :, p:p + 1152],
                                  d_rf[:, :, p:p + 1152])
            s_i4r = big.tile([128, JP, 2, M4P], F8)
            nc.sync.dma_start(s_i4r[:], d_i4r[:])
            s_csn = big.tile([M4P, 3], F16)
            nc.sync.dma_start(s_csn[:], d_csn[:])
            s_csd = big.tile([M4P, 3], F16)
            nc.sync.dma_start(s_csd[:], d_csd[:])

            # on-chip constants
            s_ones16 = big.tile([128, 128], F16)
            nc.vector.memset(s_ones16[:], 1.0)
            b_zero = big.tile([128, 1], F32)
            nc.vector.memset(b_zero[:], 0.0)
            b_eps = big.tile([128, 1], F32)
            nc.vector.memset(b_eps[:], 1e-4)
            b_neg1 = big.tile([128, 1], F32)
            nc.vector.memset(b_neg1[:], -1.0)
            b_pln16 = big.tile([128, 1], F32)
            nc.vector.memset(b_pln16[:], LN16)
            b_nln16 = big.tile([128, 1], F32)
            nc.vector.memset(b_nln16[:], -LN16)

            # ---- per-class sums + counts (col 256 is the ones column) ----
            def class_means(s_lT, s_fT, nmtag):
                ps = psS.tile([NKP, 512], F32, tag="t", name=f"ps_mean{nmtag}")
                for p in range(JP):
                    nc.tensor.matmul(ps[:, 0:F272], s_lT[:, p, :, :],
                                     s_fT[:, p, :, :], perf_mode=DR,
                                     start=(p == 0), stop=(p == JP - 1))
                cnt = small.tile([NK, 1], F32, name=f"cnt{nmtag}")
                nc.vector.tensor_copy(cnt[:], ps[0:NK, 256:257])
                rc = small.tile([NK, 1], F32, name=f"rc{nmtag}")
                nc.vector.tensor_scalar(rc[:], cnt[:], 1.0, None, ALU.max)
                nc.vector.reciprocal(rc[:], rc[:])
                meanT = work.tile([NK, C], F8, name=f"mean{nmtag}")
                nc.vector.tensor_scalar(meanT[:], ps[0:NK, 0:C], rc[:], None,
                                        ALU.mult)
                return meanT, cnt

            meanT_g, cnt_g = class_means(s_glT, s_gfT, "g")
            meanT_r, cnt_r = class_means(s_rlT, s_rfT, "r")
            vg = small.tile([NK, 1], F32)
            nc.vector.tensor_scalar(vg[:], cnt_g[:], 1.5, None, ALU.is_gt)
            valid = small.tile([NK, 1], F32)
            nc.vector.tensor_scalar(valid[:], cnt_r[:], 1.5, None, ALU.is_gt)
            nc.vector.tensor_mul(valid[:], valid[:], vg[:])

            # gl44v[(c,k), i] = gl[k, i] * valid[k]: the per-class validity
            # rides the collapse multiply, so invalid rows make BOTH the
            # numerator and denominator zero -> out = 0/0.1*0 - 1 = -1.
            def emit_gl44v():
                kcv = small.tile([NK, M4P], F8, name="kcv")
                nc.vector.tensor_scalar(kcv[:], s_kc[:], valid[:], None,
                                        ALU.mult)
                for h in range(2):
                    sl = slice(h * IW, (h + 1) * IW)
                    ps = psS.tile([M4P, 512], F32, tag="t", name="ps_gl44")
                    nc.tensor.matmul(ps[:, 0:IW], kcv[:], s_gls[:, sl],
                                     start=True, stop=True)
                    nc.vector.tensor_copy(s_gl44[:, sl], ps[:, 0:IW])
            s_gl44 = small.tile([M4P, NI], F16)

            # ---- gray side: unit16_g = 16 * (gf - mu) / ||gf - mu|| ----
            unit_g = [work.tile([128, CC, IW], F8, name="unitg0"),
                      work.tile([128, CC, IW], F8, name="unitg1")]
            for ib in range(2):
                sl = slice(ib * IW, (ib + 1) * IW)
                barg = [chk.tile([128, IW], F16, tag=f"barg{cc}", bufs=2,
                                 name=f"barg{cc}") for cc in range(CC)]
                sqg = [chk.tile([128, IW], F16, tag=f"sqg{cc}", bufs=2,
                                name=f"sqg{cc}") for cc in range(CC)]
                for cc in range(CC):
                    ps = psS.tile([128, 512], F32, tag="t", name="ps_mug")
                    nc.tensor.matmul(ps[:, 0:IW],
                                     meanT_g[:, cc * 128:(cc + 1) * 128],
                                     s_gls[:, sl], start=True, stop=True)
                    nc.any.tensor_sub(barg[cc][:], s_gfs[:, cc, sl],
                                      ps[:, 0:IW])
                    nc.any.tensor_mul(sqg[cc][:], barg[cc][:], barg[cc][:])
                ps2 = psS.tile([128, 512], F32, tag="t", name="ps_ssqg")
                for cc in range(CC):
                    nc.tensor.matmul(ps2[:, 0:IW], s_ones16[:], sqg[cc][:],
                                     start=(cc == 0), stop=(cc == CC - 1))
                lng = chk.tile([128, IW], F32, tag="lng", bufs=2, name="lng")
                nc.scalar.activation(lng[:], ps2[:, 0:IW], AF.Ln,
                                     bias=b_eps[:])
                rbg = chk.tile([128, IW], F32, tag="rbg", bufs=2, name="rbg")
                nc.scalar.activation(rbg[:], lng[:], AF.Exp,
                                     bias=b_pln16[:], scale=-0.5)
                for cc in range(CC):
                    nc.any.tensor_mul(unit_g[ib][:, cc, :], barg[cc][:],
                                      rbg[:])

            # ---- rgb side: bar_r chunks (fp8, DoubleRow layout) + per-j
            # sumsq in j-partition layout; rsqrt/16 becomes the Exp scale ----
            bar_r = {}
            # rsqrt batches: A = chunks 0-1 (jc 0-3), B = chunks 2-4
            # (jc 4-9), C = chunks 5-8 (jc 10-17)
            ssq = [small.tile([128, 2], F32, name="ssqA0"),
                   small.tile([128, 2], F32, name="ssqA1"),
                   small.tile([128, 6], F32, name="ssqB"),
                   small.tile([128, 8], F32, name="ssqC")]
            rsq = [small.tile([128, 2], F32, name="rsqA0"),
                   small.tile([128, 2], F32, name="rsqA1"),
                   small.tile([128, 6], F32, name="rsqB"),
                   small.tile([128, 8], F32, name="rsqC")]
            BASE = [0, 2, 4, 10]

            def batch_of(jc):
                bi = 0 if jc < 2 else (1 if jc < 4 else (2 if jc < 10 else 3))
                return bi, jc - BASE[bi]

            def r_chunk(ib):
                sl = slice(ib * RW, (ib + 1) * RW)
                bar8 = chk.tile([128, 2, RW], F8, tag="bar8", bufs=10,
                                name="bar8")
                sq8 = chk.tile([128, 2, RW], F16, tag="sq8", bufs=3,
                               name="sq8")
                ps = psS.tile([128, 2, RW], F32, tag="t", name="ps_mur")
                for cc in range(CC):
                    nc.tensor.matmul(ps[:, cc, :],
                                     meanT_r[:, cc * 128:(cc + 1) * 128],
                                     s_rl[:, sl], start=True, stop=True)
                nc.any.tensor_sub(bar8[:], s_rf[:, :, sl], ps[:, :, :])
                nc.any.tensor_mul(sq8[:], bar8[:], bar8[:])
                ps2 = psS.tile([128, 512], F32, tag="t", name="ps_ssqr")
                for h in range(2):
                    lo = h * 128
                    for cc in range(CC):
                        nc.tensor.matmul(ps2[:, h:h + 1],
                                         sq8[:, cc, lo:lo + 128],
                                         s_ones16[:, 0:1],
                                         start=(cc == 0), stop=(cc == CC - 1))
                bi, col = batch_of(2 * ib)
                nc.vector.tensor_copy(ssq[bi][:, col:col + 2], ps2[:, 0:2])
                bar_r[ib] = bar8

            def rsqrt_batch(bi):
                w = ssq[bi].shape[1]
                t = small.tile([128, 8], F32, name=f"lnr{bi}")
                nc.scalar.activation(t[:, 0:w], ssq[bi][:], AF.Ln,
                                     bias=b_eps[:])
                nc.scalar.activation(rsq[bi][:], t[:, 0:w], AF.Exp,
                                     bias=b_nln16[:], scale=-0.5)

            # ---- attention pairs + masked-output accumulation ----
            ps_O4K = psO.tile([M4P, 2, 512], F32)

            def attention_pair(pr):
                s_exp = expp.tile([128, 2, NI], F8, tag="exp", name="s_exp")
                for h in range(2):
                    jc = 2 * pr + h
                    ib, lo = jc // 2, (jc % 2) * 128
                    bar8 = bar_r[ib]
                    ps_mt = psM.tile([128, 2, 512], F32, tag="mt",
                                     name="ps_mt")
                    for ic in range(2):
                        nc.tensor.matmul(ps_mt[:, ic, 0:IW],
                                         bar8[:, :, lo:lo + 128],
                                         unit_g[ic][:, :, :],
                                         perf_mode=DR, start=True, stop=True)
                    bi, col = batch_of(jc)
                    nc.scalar.activation(
                        s_exp[:, h, :].rearrange("p (a b) -> p a b", a=2),
                        ps_mt[:, :, 0:IW], AF.Exp, bias=b_neg1[:],
                        scale=rsq[bi][:, col:col + 1])
                for ic in range(2):
                    i0 = ic * IW
                    nc.tensor.matmul(ps_O4K[:, ic, 0:IW], s_i4r[:, pr, :, :],
                                     s_exp[:, :, i0:i0 + IW], perf_mode=DR,
                                     start=(pr == 0), stop=(pr == JP - 1))

            # schedule: chunks 0-1 up front unlock pairs 0-1 (jc 0-3);
            # chunks 2-4 + batch B are emitted during pairs 0-1 (B is read
            # from pair 2 = jc 4); chunks 5-8 + batch C during pairs 2-4
            # (C is read from pair 5 = jc 10)
            r_chunk(0)
            rsqrt_batch(0)
            r_chunk(1)
            rsqrt_batch(1)
            NEXT = {0: [2, 3], 1: [4], 2: [5], 3: [6], 4: [7, 8]}
            for pr in range(JP):
                attention_pair(pr)
                for nxt in NEXT.get(pr, []):
                    r_chunk(nxt)
                    if nxt == 4:
                        rsqrt_batch(2)
                    if nxt == 8:
                        rsqrt_batch(3)
                if pr == 0:
                    emit_gl44v()

            # ---- finalize: class-collapse, divide by row-sum, validity ----
            # csn collapses to numerator+denominator rows 0..2; csd
            # replicates the denominator onto rows 0..2 directly.
            prod = small.tile([M4P, NI], F16)
            s_res = small.tile([3, NI], F32)
            s_rg = small.tile([3, NI], F32)
            s_rln = small.tile([3, NI], F32)
            s_rcp = small.tile([3, NI], F32)
            for h in range(2):
                sl = slice(h * IW, (h + 1) * IW)
                nc.any.tensor_mul(prod[:, sl], ps_O4K[:, h, 0:IW],
                                  s_gl44[:, sl])
                ps_nd = psS.tile([3, 512], F32, tag="t", name="ps_nd")
                nc.tensor.matmul(ps_nd[:, 0:IW], s_csn[:], prod[:, sl],
                                 start=True, stop=True)
                ps_dn = psS.tile([3, 512], F32, tag="t", name="ps_dn")
                nc.tensor.matmul(ps_dn[:, 0:IW], s_csd[:], prod[:, sl],
                                 start=True, stop=True)
                # rcp = 1/max(den, 0.1): valid rows have den >= 2*e^-2,
                # invalid ones are zeroed by rv below
                nc.any.tensor_scalar(s_rg[:, sl], ps_dn[:, 0:IW], 0.1, None,
                                     ALU.max)
                nc.scalar.activation(s_rln[:, sl], s_rg[:, sl], AF.Ln,
                                     bias=b_zero[0:3, :])
                nc.scalar.activation(s_rcp[:, sl], s_rln[:, sl], AF.Exp,
                                     bias=b_zero[0:3, :], scale=-1.0)
                # (num+den)/den = out+1; multiply by validity, subtract 1
                nc.vector.scalar_tensor_tensor(
                    s_res[:, sl], ps_nd[:, 0:IW], 1.0, s_rcp[:, sl],
                    ALU.mult, ALU.mult)
                nc.any.tensor_scalar(s_res[:, sl], s_res[:, sl], -1.0, None,
                                     ALU.add)
            nc.sync.dma_start(d_out[:], s_res[:])

    return nc


# revision 25
# speedup vs baseline: 2.3156x; 1.0596x over previous
"""Trainium2 Bass kernel for nn_C_Net_77807627534400 (sparse_attention).

Reference semantics: for each batch image and each class k in 1..11, the
per-class masked-normalized gray/rgb features form an [N,N] correlation,
softmax over the rgb-mask pixels, and a weighted mean of the rgb image is
written at the gray-mask pixels (if both masks have >= 2 pixels).

Because every pixel belongs to exactly one class, the 11 per-class [N,N]
matmuls fuse into ONE [N,N] matmul of per-class-centered features. The
class-match mask is enforced EXACTLY in the output matmul: expand img4
(rgb + ones row) to 44 rows IMG4R[(c,k), j] = img4[c,j] * rl[k,j], so

    O4K[(c,k), i] = sum_j img4[c,j] rl[k,j] e[j,i]
    O4[c, i]      = sum_k gl[k,i] O4K[(c,k), i]     (per-i class select)

with e[j,i] = exp(corr[j,i] - 1) computed WITHOUT any masking bias. The
collapse is an elementwise multiply by gl44 (gl broadcast to 44 rows via a
tiny matmul) plus two [44 -> 3] summing matmuls per half: one produces
numerator+denominator, the other the denominator replicated onto partitions
0..2 (avoids any cross-partition moves in the tail).

Normalization: gray side is explicitly normalized and scaled by 16 into
fp8 range (unit16 = 16 * bar / ||bar||); the rgb side is NOT normalized --
raw centered bar_r is the matmul operand and rsqrt(ssq_r)/16 is applied as
the per-partition *scale* of the Exp activation. Per-j sumsq is computed in
j-partition layout with tiny N=1 matmuls. All rsqrt/reciprocal come from
exp(a*ln(x) + b) so ScalarE only ever loads the natural_log_exp_and_others
table set (exp/ln/copy/square live there) -- exactly one ACT_TABLE_LOAD.

Dtypes: fp8e4 (e4m3) for every large matmul operand; the big attention and
class-means matmuls run DoubleRow (K packed 2x128, 0.5 cycles/row). PSUM is
fp32; the softmax weighted-average structure keeps fp8 quantization noise
(~6% per element, averaged over ~450 mask pixels) far below the 2e-2
tolerance. All DRAM tensors are host-side laid out to exactly match their
SBUF tiles so every DMA is contiguous.

Sharding: 8 cores = 2 batches x 4 slices of 576 gray pixels. Each core
computes the full rgb side for its batch (redundant across 4 cores) and its
576-column slice of the gray side.
"""

import numpy as np

import concourse.bass as bass
import concourse.tile as tile
from concourse import mybir
from concourse.bass_utils import run_bass_kernel_spmd
from concourse.vector_clock import ScopedClock

B, C, H, W, NCH = 2, 256, 48, 48, 12
N = H * W           # 2304
NK = NCH - 1        # classes 1..11
QS = 4              # gray-pixel slices per batch
NI = N // QS        # 576 rows per core
NCORES = B * QS     # 8
JC = N // 128       # 18 j-chunks
JP = JC // 2        # 9 j-chunk pairs
CC = C // 128       # 2 c-chunks
IW = 288            # i-chunk width (two per slice)
RW = 256            # rgb normalize chunk width
NRC = N // RW       # 9 rgb chunks
M4 = 4 * NK         # 44 expanded img rows
M4P = 48            # M4 padded so DoubleRow plane strides are 16B-aligned
NKP = 16            # NK padded likewise for the transposed labels
F272 = 272          # 258 feature+count cols padded likewise
LN16 = float(np.log(16.0))
F32 = mybir.dt.float32
F16 = mybir.dt.float16
F8 = mybir.dt.float8e4
ALU = mybir.AluOpType
AF = mybir.ActivationFunctionType
DR = mybir.MatmulPerfMode.DoubleRow


class _TC(tile.TileContext):
    """Workaround: this walrus build rejects instructions carrying more than
    one sync-wait command. Split every multi-wait instruction into a chain of
    single-wait NOPs (same engine, program order preserved) followed by the
    original instruction holding the final wait."""

    def _add_instruction(self, inst):
        si = inst.sync_info
        if si is not None:
            waits = list(si.on_wait)
            if len(waits) > 1:
                nc = self.nc
                for w in waits[:-1]:
                    nop = mybir.InstNoOp(
                        name=nc.get_next_instruction_name(),
                        sync_info=mybir.SyncInfo(on_wait=[w], on_update=[]),
                        bass_nofuse=True,
                        engine=inst.engine,
                    )
                    super()._add_instruction(nop)
                si.on_wait = waits[-1:]
                inst.sync_info = si
        super()._add_instruction(inst)

    def _drain_and_barrier(self, tick_clock, wait_clock):
        nc = self.nc
        drain_inst = nc.sync.drain()
        wait_clock.add_sem_waits(
            drain_inst.ins, ScopedClock({None: tick_clock.global_clock})
        )
        si = drain_inst.ins.sync_info
        waits = list(si.on_wait) if si is not None else []
        if len(waits) > 1:
            si.on_wait = waits[:1]
            drain_inst.ins.sync_info = si
            for w in waits[1:]:
                extra = nc.sync.drain()
                extra.ins.sync_info = mybir.SyncInfo(on_wait=[w], on_update=[])

        nc.all_engine_barrier()
        assert self.sems is not None
        popped = nc._tile_sem_poison_stack.pop()
        assert popped is self._sem_poison
        nc.clear_and_free_semaphores(list(self.sems.allocated().values()))
        nc.all_engine_barrier()


def _build_nc():
    nc = bass.Bass(target_bir_lowering=False)

    d_glT = nc.dram_tensor("glT8", [128, JP, 2, NKP], F8, kind="ExternalInput")
    d_gfT = nc.dram_tensor("gfT8", [128, JP, 2, F272], F8, kind="ExternalInput")
    d_rlT = nc.dram_tensor("rlT8", [128, JP, 2, NKP], F8, kind="ExternalInput")
    d_rfT = nc.dram_tensor("rfT8", [128, JP, 2, F272], F8, kind="ExternalInput")
    d_gls = nc.dram_tensor("gls", [NK, NI], F8, kind="ExternalInput")
    d_gfs = nc.dram_tensor("gfs", [128, CC, NI], F8, kind="ExternalInput")
    d_rl = nc.dram_tensor("rl", [NK, N], F8, kind="ExternalInput")
    d_rf = nc.dram_tensor("rf", [128, CC, N], F8, kind="ExternalInput")
    d_i4r = nc.dram_tensor("i4r", [128, JP, 2, M4P], F8, kind="ExternalInput")
    d_kc = nc.dram_tensor("kc", [NK, M4P], F8, kind="ExternalInput")
    d_csn = nc.dram_tensor("csn", [M4P, 3], F16, kind="ExternalInput")
    d_csd = nc.dram_tensor("csd", [M4P, 3], F16, kind="ExternalInput")
    d_out = nc.dram_tensor("out", [3, NI], F32, kind="ExternalOutput")

    with _TC(nc) as tc:
        with (
            tc.tile_pool(name="big", bufs=1) as big,
            tc.tile_pool(name="work", bufs=1) as work,
            tc.tile_pool(name="chk", bufs=2) as chk,
            tc.tile_pool(name="expp", bufs=3) as expp,
            tc.tile_pool(name="small", bufs=1) as small,
            tc.tile_pool(name="psS", bufs=2, space="PSUM") as psS,
            tc.tile_pool(name="psM", bufs=2, space="PSUM") as psM,
            tc.tile_pool(name="psO", bufs=1, space="PSUM") as psO,
        ):
            # ---- loads, in consumption order ----
            s_glT = big.tile([128, JP, 2, NKP], F8)
            nc.sync.dma_start(s_glT[:], d_glT[:])
            s_rlT = big.tile([128, JP, 2, NKP], F8)
            nc.sync.dma_start(s_rlT[:], d_rlT[:])
            s_gfT = big.tile([128, JP, 2, F272], F8)
            s_rfT = big.tile([128, JP, 2, F272], F8)
            for p in range(0, JP, 3):
                nc.sync.dma_start(s_gfT[:, p:p + 3, :, :], d_gfT[:, p:p + 3, :, :])
                nc.sync.dma_start(s_rfT[:, p:p + 3, :, :], d_rfT[:, p:p + 3, :, :])
            s_gls = big.tile([NK, NI], F8)
            nc.sync.dma_start(s_gls[:], d_gls[:])
            s_kc = big.tile([NK, M4P], F8)
            nc.sync.dma_start(s_kc[:], d_kc[:])
            s_gfs = big.tile([128, CC, NI], F8)
            nc.sync.dma_start(s_gfs[:], d_gfs[:])
            s_rl = big.tile([NK, N], F8)
            nc.sync.dma_start(s_rl[:], d_rl[:])
            s_rf = big.tile([128, CC, N], F8)
            for p in range(0, N, 1152):
                nc.sync.dma_start(s_rf[:, :, p:p + 1152],
                                  d_rf[:, :, p:p + 1152])
            s_i4r = big.tile([128, JP, 2, M4P], F8)
            nc.sync.dma_start(s_i4r[:], d_i4r[:])
            s_csn = big.tile([M4P, 3], F16)
            nc.sync.dma_start(s_csn[:], d_csn[:])
            s_csd = big.tile([M4P, 3], F16)
            nc.sync.dma_start(s_csd[:], d_csd[:])

            # on-chip constants
            s_ones16 = big.tile([128, 128], F16)
            nc.vector.memset(s_ones16[:], 1.0)
            b_zero = big.tile([128, 1], F32)
            nc.vector.memset(b_zero[:], 0.0)
            b_eps = big.tile([128, 1], F32)
            nc.vector.memset(b_eps[:], 1e-4)
            b_neg1 = big.tile([128, 1], F32)
            nc.vector.memset(b_neg1[:], -1.0)
            b_pln16 = big.tile([128, 1], F32)
            nc.vector.memset(b_pln16[:], LN16)
            b_nln16 = big.tile([128, 1], F32)
            nc.vector.memset(b_nln16[:], -LN16)

            # ---- per-class sums + counts (col 256 is the ones column) ----
            def class_means(s_lT, s_fT, nmtag):
                ps = psS.tile([NKP, 512], F32, tag="t", name=f"ps_mean{nmtag}")
                for p in range(JP):
                    nc.tensor.matmul(ps[:, 0:F272], s_lT[:, p, :, :],
                                     s_fT[:, p, :, :], perf_mode=DR,
                                     start=(p == 0), stop=(p == JP - 1))
                cnt = small.tile([NK, 1], F32, name=f"cnt{nmtag}")
                nc.vector.tensor_copy(cnt[:], ps[0:NK, 256:257])
                rc = small.tile([NK, 1], F32, name=f"rc{nmtag}")
                nc.vector.tensor_scalar(rc[:], cnt[:], 1.0, None, ALU.max)
                nc.vector.reciprocal(rc[:], rc[:])
                meanT = work.tile([NK, C], F8, name=f"mean{nmtag}")
                nc.vector.tensor_scalar(meanT[:], ps[0:NK, 0:C], rc[:], None,
                                        ALU.mult)
                return meanT, cnt

            meanT_g, cnt_g = class_means(s_glT, s_gfT, "g")
            meanT_r, cnt_r = class_means(s_rlT, s_rfT, "r")
            vg = small.tile([NK, 1], F32)
            nc.vector.tensor_scalar(vg[:], cnt_g[:], 1.5, None, ALU.is_gt)
            valid = small.tile([NK, 1], F32)
            nc.vector.tensor_scalar(valid[:], cnt_r[:], 1.5, None, ALU.is_gt)
            nc.vector.tensor_mul(valid[:], valid[:], vg[:])

            # gl44v[(c,k), i] = gl[k, i] * valid[k]: the per-class validity
            # rides the collapse multiply, so invalid rows make BOTH the
            # numerator and denominator zero -> out = 0/0.1*0 - 1 = -1.
            def emit_gl44v():
                kcv = small.tile([NK, M4P], F8, name="kcv")
                nc.vector.tensor_scalar(kcv[:], s_kc[:], valid[:], None,
                                        ALU.mult)
                for h in range(2):
                    sl = slice(h * IW, (h + 1) * IW)
                    ps = psS.tile([M4P, 512], F32, tag="t", name="ps_gl44")
                    nc.tensor.matmul(ps[:, 0:IW], kcv[:], s_gls[:, sl],
                                     start=True, stop=True)
                    nc.vector.tensor_copy(s_gl44[:, sl], ps[:, 0:IW])
            s_gl44 = small.tile([M4P, NI], F16)

            # ---- gray side: unit16_g = 16 * (gf - mu) / ||gf - mu|| ----
            unit_g = [work.tile([128, CC, IW], F8, name="unitg0"),
                      work.tile([128, CC, IW], F8, name="unitg1")]
            for ib in range(2):
                sl = slice(ib * IW, (ib + 1) * IW)
                barg = [chk.tile([128, IW], F16, tag=f"barg{cc}", bufs=2,
                                 name=f"barg{cc}") for cc in range(CC)]
                sqg = [chk.tile([128, IW], F16, tag=f"sqg{cc}", bufs=2,
                                name=f"sqg{cc}") for cc in range(CC)]
                for cc in range(CC):
                    ps = psS.tile([128, 512], F32, tag="t", name="ps_mug")
                    nc.tensor.matmul(ps[:, 0:IW],
                                     meanT_g[:, cc * 128:(cc + 1) * 128],
                                     s_gls[:, sl], start=True, stop=True)
                    nc.any.tensor_sub(barg[cc][:], s_gfs[:, cc, sl],
                                      ps[:, 0:IW])
                    nc.any.tensor_mul(sqg[cc][:], barg[cc][:], barg[cc][:])
                ps2 = psS.tile([128, 512], F32, tag="t", name="ps_ssqg")
                for cc in range(CC):
                    nc.tensor.matmul(ps2[:, 0:IW], s_ones16[:], sqg[cc][:],
                                     start=(cc == 0), stop=(cc == CC - 1))
                lng = chk.tile([128, IW], F32, tag="lng", bufs=2, name="lng")
                nc.scalar.activation(lng[:], ps2[:, 0:IW], AF.Ln,
                                     bias=b_eps[:])
                rbg = chk.tile([128, IW], F32, tag="rbg", bufs=2, name="rbg")
                nc.scalar.activation(rbg[:], lng[:], AF.Exp,
                                     bias=b_pln16[:], scale=-0.5)
                for cc in range(CC):
                    nc.any.tensor_mul(unit_g[ib][:, cc, :], barg[cc][:],
                                      rbg[:])

            # ---- rgb side: bar_r chunks (fp8, DoubleRow layout) + per-j
            # sumsq in j-partition layout; rsqrt/16 becomes the Exp scale ----
            bar_r = {}
            # rsqrt batches: A = chunks 0-1 (jc 0-3), B = chunks 2-4
            # (jc 4-9), C = chunks 5-8 (jc 10-17)
            ssq = [small.tile([128, 2], F32, name="ssqA0"),
                   small.tile([128, 2], F32, name="ssqA1"),
                   small.tile([128, 6], F32, name="ssqB"),
                   small.tile([128, 4], F32, name="ssqC"),
                   small.tile([128, 4], F32, name="ssqD")]
            rsq = [small.tile([128, 2], F32, name="rsqA0"),
                   small.tile([128, 2], F32, name="rsqA1"),
                   small.tile([128, 6], F32, name="rsqB"),
                   small.tile([128, 4], F32, name="rsqC"),
                   small.tile([128, 4], F32, name="rsqD")]
            BASE = [0, 2, 4, 10, 14]

            def batch_of(jc):
                bi = next(i for i in range(4, -1, -1) if jc >= BASE[i])
                return bi, jc - BASE[bi]

            def r_chunk(ib):
                sl = slice(ib * RW, (ib + 1) * RW)
                bar8 = chk.tile([128, 2, RW], F8, tag="bar8", bufs=10,
                                name="bar8")
                sq8 = chk.tile([128, 2, RW], F16, tag="sq8", bufs=3,
                               name="sq8")
                ps = psS.tile([128, 2, RW], F32, tag="t", name="ps_mur")
                for cc in range(CC):
                    nc.tensor.matmul(ps[:, cc, :],
                                     meanT_r[:, cc * 128:(cc + 1) * 128],
                                     s_rl[:, sl], start=True, stop=True)
                nc.any.tensor_sub(bar8[:], s_rf[:, :, sl], ps[:, :, :])
                nc.any.tensor_mul(sq8[:], bar8[:], bar8[:])
                ps2 = psS.tile([128, 512], F32, tag="t", name="ps_ssqr")
                for h in range(2):
                    lo = h * 128
                    for cc in range(CC):
                        nc.tensor.matmul(ps2[:, h:h + 1],
                                         sq8[:, cc, lo:lo + 128],
                                         s_ones16[:, 0:1],
                                         start=(cc == 0), stop=(cc == CC - 1))
                bi, col = batch_of(2 * ib)
                nc.vector.tensor_copy(ssq[bi][:, col:col + 2], ps2[:, 0:2])
                bar_r[ib] = bar8

            def rsqrt_batch(bi):
                w = ssq[bi].shape[1]
                t = small.tile([128, 8], F32, name=f"lnr{bi}")
                nc.scalar.activation(t[:, 0:w], ssq[bi][:], AF.Ln,
                                     bias=b_eps[:])
                nc.scalar.activation(rsq[bi][:], t[:, 0:w], AF.Exp,
                                     bias=b_nln16[:], scale=-0.5)

            # ---- attention pairs + masked-output accumulation ----
            ps_O4K = psO.tile([M4P, 2, 512], F32)

            def attention_pair(pr):
                s_exp = expp.tile([128, 2, NI], F8, tag="exp", name="s_exp")
                for h in range(2):
                    jc = 2 * pr + h
                    ib, lo = jc // 2, (jc % 2) * 128
                    bar8 = bar_r[ib]
                    ps_mt = psM.tile([128, 2, 512], F32, tag="mt",
                                     name="ps_mt")
                    for ic in range(2):
                        nc.tensor.matmul(ps_mt[:, ic, 0:IW],
                                         bar8[:, :, lo:lo + 128],
                                         unit_g[ic][:, :, :],
                                         perf_mode=DR, start=True, stop=True)
                    bi, col = batch_of(jc)
                    nc.scalar.activation(
                        s_exp[:, h, :].rearrange("p (a b) -> p a b", a=2),
                        ps_mt[:, :, 0:IW], AF.Exp, bias=b_neg1[:],
                        scale=rsq[bi][:, col:col + 1])
                for ic in range(2):
                    i0 = ic * IW
                    nc.tensor.matmul(ps_O4K[:, ic, 0:IW], s_i4r[:, pr, :, :],
                                     s_exp[:, :, i0:i0 + IW], perf_mode=DR,
                                     start=(pr == 0), stop=(pr == JP - 1))

            # schedule: chunks 0-1 up front unlock pairs 0-1 (jc 0-3);
            # chunks 2-4 + batch B are emitted during pairs 0-1 (B is read
            # from pair 2 = jc 4); chunks 5-8 + batch C during pairs 2-4
            # (C is read from pair 5 = jc 10)
            r_chunk(0)
            rsqrt_batch(0)
            r_chunk(1)
            rsqrt_batch(1)
            NEXT = {0: [2, 3], 1: [4], 2: [5], 3: [6], 4: [7], 5: [8]}
            for pr in range(JP):
                attention_pair(pr)
                for nxt in NEXT.get(pr, []):
                    r_chunk(nxt)
                    if nxt == 4:
                        rsqrt_batch(2)
                    if nxt == 6:
                        rsqrt_batch(3)
                    if nxt == 8:
                        rsqrt_batch(4)
                if pr == 0:
                    emit_gl44v()

            # ---- finalize: class-collapse, divide by row-sum, validity ----
            # csn collapses to numerator+denominator rows 0..2; csd
            # replicates the denominator onto rows 0..2 directly.
            prod = small.tile([M4P, NI], F16)
            s_res = small.tile([3, NI], F32)
            s_rg = small.tile([3, NI], F32)
            s_rln = small.tile([3, NI], F32)
            s_rcp = small.tile([3, NI], F32)
            for h in range(2):
                sl = slice(h * IW, (h + 1) * IW)
                nc.any.tensor_mul(prod[:, sl], ps_O4K[:, h, 0:IW],
                                  s_gl44[:, sl])
                ps_nd = psS.tile([3, 512], F32, tag="t", name="ps_nd")
                nc.tensor.matmul(ps_nd[:, 0:IW], s_csn[:], prod[:, sl],
                                 start=True, stop=True)
                ps_dn = psS.tile([3, 512], F32, tag="t", name="ps_dn")
                nc.tensor.matmul(ps_dn[:, 0:IW], s_csd[:], prod[:, sl],
                                 start=True, stop=True)
                # rcp = 1/max(den, 0.1): valid rows have den >= 2*e^-2,
                # invalid ones are zeroed by rv below
                nc.any.tensor_scalar(s_rg[:, sl], ps_dn[:, 0:IW], 0.1, None,
                                     ALU.max)
                nc.scalar.activation(s_rln[:, sl], s_rg[:, sl], AF.Ln,
                                     bias=b_zero[0:3, :])
                nc.scalar.activation(s_rcp[:, sl], s_rln[:, sl], AF.Exp,
                                     bias=b_zero[0:3, :], scale=-1.0)
                # (num+den)/den = out+1; multiply by validity, subtract 1
                nc.vector.scalar_tensor_tensor(
                    s_res[:, sl], ps_nd[:, 0:IW], 1.0, s_rcp[:, sl],
                    ALU.mult, ALU.mult)
                nc.vector.tensor_scalar(s_res[:, sl], s_res[:, sl], -1.0,
                                        None, ALU.add)
            nc.sync.dma_start(d_out[:], s_res[:])

    return nc
